# revision 32
# baseline (speedup 1.0000x reference)
"""Trainium2 Bass kernel for nn_MOA_13254269075617 (sparse windowed attention block).

Sharding: data-parallel over batch B=8 across 8 NeuronCores (1 image each).
BatchNorm uses global batch stats via an on-device AllReduce of per-channel
sum / sum-of-squares.

Per-core pipeline (all in the spatially-TRANSPOSED frame; host pre-permutes
the input to token order l' = w*64 + h and un-permutes the c-major output):
  xT_cm  : x channel-major [2x128, 4096] via transpose-DMAs
  vT     : (x @ Wv + bv) token-major [128, 32, 256] bf16
  A      : softmax(x @ Wa + ba) pq-major [81, 4096] on a zero-padded grid
  W_tap  : 25-tap position-varying stencil weights [25, 4096] (fold+attention
           combined algebraically), wmask'd at grid edges
  G      : banded token->token weight matrices assembled in DRAM by strided
           scatter-DMA (5-tap f-runs = 10B descriptors), stored transposed
           [m, k] and un-transposed by the DMA XBAR on load
  xf     : stencil apply = 6 PE matmuls per 128-token chunk,
           out[c, m] = sum_k vT[k, c] * G[k, m], PSUM-accumulated c-major
  x1/x2  : relu chains with 3x3/5x5 maxpools (separable shifted-max trees)
  out    : concat-matmul (Wfu) + residual, BN with AllReduce'd stats,
           written c-major [256, 4096]; host transposes back
"""
import sys

for _p in (
    "/root/.axon_site",
    "/root/.axon_site/_ro/trn_rl_repo",
    "/root/.axon_site/_ro/pypackages",
    "/opt/trn_rl_repo",
):
    if _p not in sys.path:
        sys.path.append(_p)

from itertools import product

import numpy as np

import concourse.bass as bass
import concourse.tile as tile
from concourse.ap import AP
from concourse import bacc, mybir
from concourse.bass_utils import run_bass_kernel_spmd

F32 = mybir.dt.float32
BF16 = mybir.dt.bfloat16
ALU = mybir.AluOpType
ACT = mybir.ActivationFunctionType

import os
DEBUG2 = os.environ.get("KDEBUG") == "1"
B, H, W, C = 8, 64, 64, 256
L = H * W                      # 4096 tokens
NCHUNK = L // 128              # 32 token chunks
N_CORES = 8
EPS = 1e-5
GROW = 384                     # 3 source blocks x 128 rows per chunk
GJ = GROW * 128                # G elems per chunk
GJ2 = GJ + 128                 # chunk stride in gs: +128 so row images of
                               # adjacent chunks never overlap (no WAR chains)
GSPAD = NCHUNK * GJ2 + GJ + 4096  # guard for conservative OOB checks
DEBUG = False
TAPS = [(e, f) for e in range(-2, 3) for f in range(-2, 3)]


def host_consts():
    """Selector matrices and small constants (host-precomputed, same all cores)."""
    selsum = np.zeros((81, 9), np.float32)
    for p in range(9):
        selsum[9 * p:9 * p + 9, p] = 1.0
    selrep = np.zeros((9, 81), np.float32)
    for p in range(9):
        selrep[p, 9 * p:9 * p + 9] = 1.0
    # selshift[:, 25*d + tap]: for (di,dj) block d, tap (e,f):
    #   k = 9*(3di+dj) + 3(di+e)+(dj+f) if di+e,dj+f in [0,3)
    selshift = np.zeros((81, 9 * 25), np.float32)
    for d, (di, dj) in enumerate(product(range(3), range(3))):
        for t, (e, f) in enumerate(product(range(-2, 3), range(-2, 3))):
            dip, djp = di + e, dj + f
            if 0 <= dip < 3 and 0 <= djp < 3:
                k = 9 * (3 * di + dj) + (3 * dip + djp)
                selshift[k, 25 * d + t] = 1.0
    wmask = np.ones((25, 64, 64), np.float32)
    for t, (e, f) in enumerate(product(range(-2, 3), range(-2, 3))):
        if e > 0: wmask[t, 64 - e:, :] = 0
        if e < 0: wmask[t, :-e, :] = 0
        if f > 0: wmask[t, :, 64 - f:] = 0
        if f < 0: wmask[t, :, :-f] = 0
    import ml_dtypes
    return {
        "selsum": selsum,
        "selrep": selrep,
        "selshift": selshift,
        "wmask": wmask.reshape(25, 4096),
        "ident25": np.eye(25, dtype=np.float32),
        "ones1": np.ones((1, 128), np.float32),
        "gs": np.zeros(GSPAD, dtype=ml_dtypes.bfloat16),
    }


def build(nc, n_cores):
    d = {}
    def din(name, shape):
        d[name] = nc.dram_tensor(name, list(shape), F32, kind="ExternalInput").ap()

    d["xbf"] = nc.dram_tensor("xbf", [L, C], BF16, kind="ExternalInput").ap()
    d["gs"] = nc.dram_tensor("gs", [GSPAD], BF16, kind="ExternalInput").ap()
    din("wv", (C, C)); din("bv", (1, C))
    din("wa", (C, 81)); din("ba", (81, 1))
    din("wfu", (2 * C, C)); din("bfu2", (128, 2))
    din("gamma2", (128, 2)); din("beta2", (128, 2))
    din("selsum", (81, 9)); din("selrep", (9, 81)); din("selshift", (81, 225))
    din("ident25", (25, 25)); din("ones1", (1, 128)); din("wmask", (25, L))
    d["y"] = nc.dram_tensor("y", [2 * 128, L], BF16, kind="ExternalOutput").ap()
    if DEBUG2:
        d["dbg_vt"] = nc.dram_tensor("dbg_vt", [128, NCHUNK * C], BF16, kind="ExternalOutput").ap()
        d["dbg_wtap"] = nc.dram_tensor("dbg_wtap", [25, L], BF16, kind="ExternalOutput").ap()
        d["dbg_wtm"] = nc.dram_tensor("dbg_wtm", [128, NCHUNK * 25], BF16, kind="ExternalOutput").ap()
        d["dbg_ae"] = nc.dram_tensor("dbg_ae", [81, 66 * 67], BF16, kind="ExternalOutput").ap()
        d["dbg_gs"] = nc.dram_tensor("dbg_gs", [NCHUNK * GJ2], BF16, kind="ExternalOutput").ap()
        d["dbg_out"] = nc.dram_tensor("dbg_out", [128, 2 * L], F32, kind="ExternalOutput").ap()
        d["dbg_xcm"] = nc.dram_tensor("dbg_xcm", [128, 2 * L], BF16, kind="ExternalOutput").ap()
    if DEBUG:
        d["dbg_wtap"] = nc.dram_tensor("dbg_wtap", [25, L], F32, kind="ExternalOutput").ap()
        d["dbg_wtm"] = nc.dram_tensor("dbg_wtm", [128, NCHUNK * 25], F32, kind="ExternalOutput").ap()
        d["dbg_wtmd"] = nc.dram_tensor("dbg_wtmd", [L * 25], BF16, kind="ExternalOutput").ap()
        d["dbg_gs"] = nc.dram_tensor("dbg_gs", [4 * GJ], BF16, kind="ExternalOutput").ap()
        d["dbg_x1"] = nc.dram_tensor("dbg_x1", [2 * 128, L], F32, kind="ExternalOutput").ap()
        d["dbg_vt"] = nc.dram_tensor("dbg_vt", [128, NCHUNK * C], F32, kind="ExternalOutput").ap()

    with tile.TileContext(nc) as tc:
        _build_tc(tc, d, n_cores)
    return d


def _build_tc(tc, d, n_cores):
    nc = tc.nc
    from contextlib import ExitStack
    es = ExitStack()
    with es:
        consts = es.enter_context(tc.tile_pool(name="consts", bufs=1))
        main = es.enter_context(tc.tile_pool(name="main", bufs=1))
        gpool = es.enter_context(tc.tile_pool(name="gpool", bufs=8))
        dram = es.enter_context(tc.tile_pool(name="dram", bufs=2, space="DRAM"))

        # ---- const loads ----
        def cload(name, shape):
            t = consts.tile(list(shape), F32, tag=name, name=name)
            nc.gpsimd.dma_start(t[:], d[name][:])
            return t
        def cload_bf(name, shape):
            t = consts.tile(list(shape), BF16, tag=name, name=name)
            nc.gpsimd.dma_start(t[:], d[name][:])
            return t
        # order by first use: C needs wa/ba, B needs wv/bv/ones1, then
        # softmax/D selectors, then late-phase consts
        wa_sb = consts.tile([128, 2, 81], BF16, tag="wa", name="wa_sb")
        for kc in range(2):
            nc.gpsimd.dma_start(wa_sb[:, kc, :], d["wa"][128 * kc:128 * (kc + 1), :])
        ba_sb = cload("ba", (81, 1))
        wv_sb = consts.tile([128, 2, C], BF16, tag="wv", name="wv_sb")
        for kc in range(2):
            nc.gpsimd.dma_start(wv_sb[:, kc, :], d["wv"][128 * kc:128 * (kc + 1), :])
        ones1 = cload_bf("ones1", (1, 128))
        bv_sb = cload_bf("bv", (1, C))
        selsum = cload_bf("selsum", (81, 9))
        selrep_bf = cload_bf("selrep", (9, 81))
        selshift = cload_bf("selshift", (81, 225))
        ident25 = cload_bf("ident25", (25, 25))
        wmask = main.tile([25, L], BF16, tag="wmask", name="wmask")
        nc.gpsimd.dma_start(wmask[:], d["wmask"][:])
        wfu_sb = consts.tile([128, 4, 2, 128], BF16, tag="wfu", name="wfu_sb")
        for kc in range(4):
            for mc in range(2):
                nc.gpsimd.dma_start(
                    wfu_sb[:, kc, mc, :],
                    d["wfu"][128 * kc:128 * (kc + 1), 128 * mc:128 * (mc + 1)])
        bfu2 = cload("bfu2", (128, 2))
        gamma2 = cload("gamma2", (128, 2))
        beta2 = cload("beta2", (128, 2))

        # ---- phase A: transpose-DMA x straight into channel-major ----
        # host pre-permuted xbf rows to l' = w*64 + h, so no on-chip permute
        xT_cm = [main.tile([128, L], BF16, tag=f"xcm{cc}", name=f"xT_cm{cc}")
                 for cc in range(2)]
        # ALL transpose-DMAs go on ONE queue: two concurrent transposes on
        # different queues interleave in the shared XBAR and corrupt tiles
        # (seen as even/odd-channel, stride-16-token garbage).
        a_dmas = []
        for cc in range(2):
            for q in range(8):
                a_dmas.append(nc.sync.dma_start_transpose(
                    xT_cm[cc][:, 512 * q:512 * (q + 1)],
                    d["xbf"][512 * q:512 * (q + 1), 128 * cc:128 * (cc + 1)]))

        # ---- phase B: vT = xT @ Wv + bv, token-major (bf16) ----
        cmAB = tc.tile_pool(name="psAB", bufs=3, space="PSUM"); psAB = cmAB.__enter__()
        vT = main.tile([128, NCHUNK, C], BF16, tag="vT", name="vT")
        bv256 = consts.tile([128, C], BF16, tag="bv256", name="bv256")
        psb = psAB.tile([128, C], F32, tag="vps", name="vps")
        nc.tensor.matmul(psb[:], ones1[:], bv_sb[:], start=True, stop=True)
        nc.scalar.copy(bv256[:], psb[:])
        # XBAR transpose-DMA completion semaphores fire before all packets
        # land, so neither tracker edges nor direct deps on the transposes are
        # safe.  Barrier: a small STANDARD DMA per queue issued after the
        # transposes (same-ring FIFO -> its accurate completion bounds their
        # drain), then a vector probe, then a PE NOP so not even a LDWEIGHTS
        # prefetch can read xT_cm early.
        from concourse.tile import add_dep_helper as _adh
        guard_t = consts.tile([1, 4], F32, tag="guard", name="guard")
        gprobe = nc.vector.memset(guard_t[:, 0:1], 0.0)
        for dma in a_dmas:
            _adh(gprobe.ins, dma.ins, reason="x loads complete")
        pe_guard_b = nc.tensor.nop(nofuse=True, hint="B xT_cm LDW guard")
        _adh(pe_guard_b.ins, gprobe.ins, reason="B LDW prefetch guard")
        for j in range(NCHUNK):
            ps = psAB.tile([128, C], F32, tag="vps", name="vps")
            mmb = nc.tensor.matmul(ps[:], xT_cm[0][:, 128 * j:128 * (j + 1)],
                                   wv_sb[:, 0, :], start=True, stop=False)
            if j == 0:
                _adh(mmb.ins, pe_guard_b.ins, reason="B LDW prefetch guard")
            nc.tensor.matmul(ps[:], xT_cm[1][:, 128 * j:128 * (j + 1)],
                             wv_sb[:, 1, :], start=False, stop=True)
            nc.vector.tensor_tensor(vT[:, j, :], ps[:], bv256[:], op=ALU.add)

        # ---- phase C: attention logits -> exp -> normalize ----
        cmAB.__exit__(None, None, None)
        cmC = tc.tile_pool(name="psC", bufs=2, space="PSUM"); psC = cmC.__enter__()
        AE = main.tile([81, 66 * 67], BF16, tag="AE", name="AE")
        nc.gpsimd.memset(AE[:], 0.0)
        AE3 = AE.rearrange("p (r s) -> p r s", r=67)

        # NOTE: the automatic dependency tracker is unreliable for the strided
        # AE3 views, so cross-engine RAW edges here are added explicitly
        # (per-engine in-order execution covers the downstream instructions).
        from concourse.tile import add_dep_helper
        exp_insts = []
        for n8 in range(8):
            ps = psC.tile([81, 512], F32, tag="aps", name="aps")
            for kc in range(2):
                nc.tensor.matmul(ps[:], wa_sb[:, kc, :],
                                 xT_cm[kc][:, 512 * n8:512 * (n8 + 1)],
                                 start=(kc == 0), stop=(kc == 1))
            exp_insts.append(nc.scalar.activation(
                AE3[:, 1 + 8 * n8:1 + 8 * n8 + 8, 1:65],
                ps.rearrange("p (r s) -> p r s", s=64),
                ACT.Exp, bias=ba_sb[:, 0:1]))
        ROWCH = [(r0, min(7, 64 - r0)) for r0 in range(0, 64, 7)]
        norm_insts = []
        for r0, nr in ROWCH:
            N = nr * 66
            win = slice((r0 + 1) * 66, (r0 + 1) * 66 + N)
            ps = psC.tile([9, 512], F32, tag="sps", name="sps")
            mm = nc.tensor.matmul(ps[:, 0:N], selsum[:], AE[:, win],
                                  start=True, stop=True)
            # rowsum reads AE rows [r0+1, r0+1+nr): wait for the exp blocks
            for n8 in range(max(0, r0 // 8), min(8, (r0 + nr) // 8 + 1)):
                add_dep_helper(mm.ins, exp_insts[n8].ins,
                               reason="rowsum reads exp'd AE rows")
            rchf = consts.tile([9, 512], F32, tag="rchunkf", name="rchf", bufs=1)
            nc.vector.reciprocal_approx_fast(rchf[:, 0:N], ps[:, 0:N])
            rch = consts.tile([9, 512], BF16, tag="rchunk", name="rch", bufs=1)
            nc.scalar.copy(rch[:, 0:N], rchf[:, 0:N])
            ps2 = psC.tile([81, 512], F32, tag="rps", name="rps")
            nc.tensor.matmul(ps2[:, 0:N], selrep_bf[:], rch[:, 0:N],
                             start=True, stop=True)
            iv = AE3[:, r0 + 1:r0 + 1 + nr, 1:65]
            nv = nc.vector.tensor_tensor(
                iv, iv, ps2[:, 0:N].rearrange("p (r s) -> p r s", s=66)[:, :, 1:65],
                op=ALU.mult)
            for n8 in range(max(0, r0 // 8), min(8, (r0 + nr) // 8 + 1)):
                add_dep_helper(nv.ins, exp_insts[n8].ins,
                               reason="normalize RMWs exp'd AE rows")
            norm_insts.append(nv)

        # ---- phase D: W stencil build (9 shifted selector matmuls) ----
        cmC.__exit__(None, None, None)
        cmD = tc.tile_pool(name="psD", bufs=8, space="PSUM"); psD = cmD.__enter__()
        W_tap = main.tile([25, L], BF16, tag="wtap", name="W_tap")
        wmask_t = wmask.rearrange("p (u v) -> p v u", u=64)
        wtap_t = W_tap.rearrange("p (u v) -> p v u", u=64)
        ev_insts = []
        first_d = True
        for r0, nr in ROWCH:
            N = nr * 66
            ps = psD.tile([25, 512], F32, tag="wps", name="wps")
            for dd, (di, dj) in enumerate(product(range(3), range(3))):
                st = (r0 + 2 - dj) * 66 + (2 - di)
                mm = nc.tensor.matmul(ps[:, 0:N],
                                      selshift[:, 25 * dd:25 * (dd + 1)],
                                      AE[:, st:st + N],
                                      start=(dd == 0), stop=(dd == 8))
                if first_d:
                    # PE is in-order: gating the first D matmul on all
                    # normalizes covers every later AE read in phase D
                    for nv in norm_insts:
                        add_dep_helper(mm.ins, nv.ins,
                                       reason="D reads normalized AE")
                    first_d = False
            ev_insts.append(nc.vector.tensor_tensor(
                wtap_t[:, r0:r0 + nr, :],
                ps[:, 0:N].rearrange("p (r s) -> p r s", s=66)[:, :, 0:64],
                wmask_t[:, r0:r0 + nr, :], op=ALU.mult))
        cmD.__exit__(None, None, None)

        # ---- phase D2: transpose W to token-major (SBUF only) ----
        cmD2 = tc.tile_pool(name="psD2", bufs=3, space="PSUM"); psD2 = cmD2.__enter__()
        W_tm = main.tile([128, NCHUNK, 25], BF16, tag="wtm", name="W_tm")
        if DEBUG:
            nc.gpsimd.dma_start(d["dbg_wtap"][:], W_tap[:])
        # LDWEIGHTS prefetch hazard: tr's stationary is W_tap, whose writes go
        # through a strided view the tracker misses, and a dep ON tr gates the
        # matmul entry, not its LDW (which executes before the wait).  Gate a
        # PE NOP queue entry on the evacs so no LDW can read W_tap early.
        pe_guard = nc.tensor.nop(nofuse=True, hint="D2 W_tap guard")
        for ev in ev_insts:
            add_dep_helper(pe_guard.ins, ev.ins, reason="D2 reads W_tap")
        wtm_copies = []
        for j in range(NCHUNK):
            pt = psD2.tile([128, 25], BF16, tag="wtp", name="wtp")
            tr = nc.tensor.transpose(pt[:], W_tap[:, 128 * j:128 * (j + 1)],
                                     ident25[:])
            if j == 0:
                add_dep_helper(tr.ins, pe_guard.ins, reason="D2 after guard")
            wtm_copies.append(nc.scalar.copy(W_tm[:, j, :], pt[:]))
        cmD2.__exit__(None, None, None)
        # ---- G^T row-image build, one plain strided store per chunk ----
        # GT[j][m, k] = weight linking source token 128*(j + k//128 - 1) + k%128
        # to output token 128*j + m; tap (e,f) occupies k = m + 64e + f + 128,
        # i.e. flat position 385*m + 64e + f + 128 (+2 global shift so row
        # windows are non-negative).  Row m's 25 taps live at in-row offsets
        # 64*(e+2) + (f+2) of a [128, 384] SBUF image whose gaps are zero, so
        # one strided SBUF copy + one 2D-strided DRAM store per chunk replaces
        # the diagonal scatter.  Geometric clipping (k outside [0, 384)) is
        # exactly the set of taps wmask already zeroed, so no edge cases.
        # The store dst/load src are plain 2D APs the dependency tracker can
        # range-analyze, giving completion-accurate store->load edges.
        gs_t = d["gs"].tensor
        engs = [nc.sync, nc.scalar]
        probe_t = consts.tile([1, 8], F32, tag="probe", name="probe")

        # ---- phase F: maxpools on xT_cm (channel-major grid) ----
        ptmp = es.enter_context(tc.tile_pool(name="ptmp", bufs=3))
        mp_copies = []
        m1 = [main.tile([128, L], BF16, tag=f"m1{cc}", name=f"m1_{cc}") for cc in range(2)]
        m2 = [main.tile([128, L], BF16, tag=f"m2{cc}", name=f"m2_{cc}") for cc in range(2)]

        def g3(ap):
            return ap.rearrange("p (h w) -> p h w", h=64)

        def hmax3(eng, dst, src):
            dv, sv = g3(dst), g3(src)
            t1 = ptmp.tile([128, L], BF16, tag="ptmp", name="ptmp")
            tv = g3(t1)
            eng.tensor_tensor(tv[:, :, 1:], sv[:, :, 1:], sv[:, :, :63], op=ALU.max)
            mp_copies.append(nc.scalar.copy(tv[:, :, 0:1], sv[:, :, 0:1]))
            eng.tensor_tensor(dv[:, :, :63], tv[:, :, :63], sv[:, :, 1:], op=ALU.max)
            mp_copies.append(nc.scalar.copy(dv[:, :, 63:64], tv[:, :, 63:64]))

        def vmax3(eng, dst, src):
            dv, sv = g3(dst), g3(src)
            t1 = ptmp.tile([128, L], BF16, tag="ptmp", name="ptmp")
            tv = g3(t1)
            eng.tensor_tensor(tv[:, 1:, :], sv[:, 1:, :], sv[:, :63, :], op=ALU.max)
            mp_copies.append(nc.scalar.copy(tv[:, 0:1, :], sv[:, 0:1, :]))
            eng.tensor_tensor(dv[:, :63, :], tv[:, :63, :], sv[:, 1:, :], op=ALU.max)
            mp_copies.append(nc.scalar.copy(dv[:, 63:64, :], tv[:, 63:64, :]))

        def hspread(eng, dst, src):   # dst[v] = max(src[v-1], src[v+1]) + edge copies
            dv, sv = g3(dst), g3(src)
            eng.tensor_tensor(dv[:, :, 1:63], sv[:, :, 0:62], sv[:, :, 2:64], op=ALU.max)
            mp_copies.append(nc.scalar.copy(dv[:, :, 0:1], sv[:, :, 1:2]))
            mp_copies.append(nc.scalar.copy(dv[:, :, 63:64], sv[:, :, 62:63]))

        def vspread(eng, dst, src):
            dv, sv = g3(dst), g3(src)
            eng.tensor_tensor(dv[:, 1:63, :], sv[:, 0:62, :], sv[:, 2:64, :], op=ALU.max)
            mp_copies.append(nc.scalar.copy(dv[:, 0:1, :], sv[:, 1:2, :]))
            mp_copies.append(nc.scalar.copy(dv[:, 63:64, :], sv[:, 62:63, :]))

        for cc in range(2):
            eng = nc.vector
            cm3 = ptmp.tile([128, L], BF16, tag="ptmp", name="ptmp")
            hmax3(eng, cm3, xT_cm[cc])
            vmax3(eng, m1[cc], cm3)
            cm5 = ptmp.tile([128, L], BF16, tag="ptmp", name="ptmp")
            hspread(eng, cm5, cm3)
            r35 = ptmp.tile([128, L], BF16, tag="ptmp", name="ptmp")
            vmax3(eng, r35, cm5)
            vspread(eng, m2[cc], r35)

        # ---- phase E: banded stencil apply, c-major out ----
        cmE = tc.tile_pool(name="psE", bufs=4, space="PSUM"); psE = cmE.__enter__()
        x1 = [main.tile([128, L], BF16, tag=f"x1{cc}", name=f"x1_{cc}") for cc in range(2)]
        x2 = [main.tile([128, L], BF16, tag=f"x2{cc}", name=f"x2_{cc}") for cc in range(2)]
        # ONE XBAR load per chunk: G rows (m, b) interleave contiguously at
        # stride 128 (= the XBAR column count), i.e. the transpose input is a
        # plain contiguous [384, 128] block. Column m of source block b then
        # sits at rhs position 3*m + b (stride-3 moving-operand AP).
        last_mm_of_chunk = {}
        gst_of_chunk = {}
        for j in range(NCHUNK):
            # build the [128, 384] zero-gapped row image for chunk j and
            # store it as one 2D-strided DMA (row m -> gs[j*GJ + 385*m ..])
            gsb = gpool.tile([128, 384], BF16, tag="gsb", name="gsb", bufs=4)
            if j < 4:
                nc.vector.memset(gsb[:], 0.0)
            cp = nc.vector.tensor_scalar(
                gsb.rearrange("p (g c) -> p g c", c=64)[:, 0:5, 0:5],
                W_tm[:, j, :].rearrange("p (g f) -> p g f", f=5),
                1.0, None, op0=ALU.mult)
            add_dep_helper(cp.ins, wtm_copies[j].ins, reason="gsb reads W_tm")
            if j - 4 in gst_of_chunk:
                add_dep_helper(cp.ins, gst_of_chunk[j - 4].ins,
                               reason="gsb buffer WAR vs store")
            st = nc.scalar.dma_start(
                AP(tensor=gs_t, offset=j * GJ2, ap=[[385, 128], [1, 261]]),
                gsb[:, 0:261])
            add_dep_helper(st.ins, cp.ins, reason="store reads gsb")
            gst_of_chunk[j] = st
            g = gpool.tile([128, 384], BF16, tag="g", name="g")
            g3v = g.rearrange("k (m b) -> k m b", b=3)
            ld = nc.sync.dma_start_transpose(
                g[:],
                AP(tensor=gs_t, offset=j * GJ2 + 2,
                   ap=[[128, 384], [1, 128]]))
            add_dep_helper(ld.ins, st.ins, reason="G load after store")
            # WAR: this load reuses the g buffer read by matmuls 8 chunks back
            if j - 8 in last_mm_of_chunk:
                add_dep_helper(ld.ins, last_mm_of_chunk[j - 8].ins,
                               reason="g-buffer WAR")
            bs = [b for b in range(3) if 0 <= j + b - 1 < NCHUNK]
            first_of_chunk = True
            for cc in range(2):
                psx = psE.tile([128, 128], F32, tag=f"psx{cc}", name="psx")
                for i, b in enumerate(bs):
                    mm = nc.tensor.matmul(
                        psx[:],
                        vT[:, j + b - 1, 128 * cc:128 * (cc + 1)],
                        g3v[:, :, b],
                        start=(i == 0), stop=(i == len(bs) - 1))
                    if first_of_chunk:
                        add_dep_helper(mm.ins, ld.ins,
                                       reason="E matmul reads G load")
                        first_of_chunk = False
                last_mm_of_chunk[j] = mm
                nc.scalar.activation(x1[cc][:, 128 * j:128 * (j + 1)],
                                     psx[:], ACT.Relu)
        cmE.__exit__(None, None, None)

        if DEBUG:
            for cc in range(2):
                nc.gpsimd.dma_start(d["dbg_x1"][128 * cc:128 * (cc + 1), :], x1[cc][:])
            nc.gpsimd.dma_start(
                d["dbg_vt"][:].rearrange("p (j c) -> p j c", c=C), vT[:])

        # ---- phase G tail: x1 = relu(xr + m1); x2 = relu(x1 + m2) ----
        first_gt = True
        for n8 in range(8):
            sl = slice(512 * n8, 512 * (n8 + 1))
            for cc in range(2):
                gt = nc.vector.tensor_tensor(x1[cc][:, sl], x1[cc][:, sl],
                                             m1[cc][:, sl], op=ALU.add)
                if first_gt:
                    for cp in mp_copies:
                        add_dep_helper(gt.ins, cp.ins,
                                       reason="m-add reads maxpool edge fills")
                    first_gt = False
                nc.scalar.activation(x1[cc][:, sl], x1[cc][:, sl], ACT.Relu)
                nc.vector.tensor_tensor(x2[cc][:, sl], x1[cc][:, sl],
                                        m2[cc][:, sl], op=ALU.add)
                nc.scalar.activation(x2[cc][:, sl], x2[cc][:, sl], ACT.Relu)

        # ---- phase H: fu matmul + residual (mc-outer), BN per half ----
        cmH = tc.tile_pool(name="psH", bufs=4, space="PSUM"); psH = cmH.__enter__()
        out_all = main.tile([128, 2, L], F32, tag="out", name="out_all")
        out_cm = [out_all[:, cc, :] for cc in range(2)]
        small = es.enter_context(tc.tile_pool(name="small", bufs=1))
        bnpack = small.tile([128, 4], F32, tag="bnpack", name="bnpack")
        cin = dram.tile([128, 4], F32, name="cin")
        cout = dram.tile([128, 4], F32, name="cout")
        rhss = [x1[0], x1[1], x2[0], x2[1]]
        for mc in range(2):
            for n8 in range(8):
                sl = slice(512 * n8, 512 * (n8 + 1))
                ps = psH.tile([128, 512], F32, tag="fups", name="fups")
                for kc in range(4):
                    nc.tensor.matmul(ps[:], wfu_sb[:, kc, mc, :],
                                     rhss[kc][:, sl],
                                     start=(kc == 0), stop=(kc == 3))
                nc.scalar.activation(out_cm[mc][:, sl], ps[:], ACT.Relu,
                                     bias=bfu2[:, mc:mc + 1])
                nc.vector.tensor_tensor(out_cm[mc][:, sl], out_cm[mc][:, sl],
                                        xT_cm[mc][:, sl], op=ALU.add)
            st = small.tile([128, 8, 6], F32, tag="bnst", name="bnst")
            for n8 in range(8):
                nc.vector.bn_stats(st[:, n8, :], out_cm[mc][:, 512 * n8:512 * (n8 + 1)])
            ag = small.tile([128, 2], F32, tag="bnag", name="bnag")
            nc.vector.bn_aggr(ag[:], st[:])
            nc.vector.tensor_scalar(bnpack[:, 2 * mc:2 * mc + 1], ag[:, 0:1],
                                    float(L), None, op0=ALU.mult)
            sq = small.tile([128, 1], F32, tag="bnsq", name="bnsq")
            nc.vector.tensor_tensor(sq[:], ag[:, 0:1], ag[:, 0:1], op=ALU.mult)
            nc.vector.tensor_tensor(sq[:], sq[:], ag[:, 1:2], op=ALU.add)
            nc.vector.tensor_scalar(bnpack[:, 2 * mc + 1:2 * mc + 2], sq[:],
                                    float(L), None, op0=ALU.mult)
        # one packed AllReduce for both halves' (sum, sumsq)
        nc.sync.dma_start(cin[:], bnpack[:])
        nc.gpsimd.collective_compute(
            "AllReduce", ALU.add,
            replica_groups=[list(range(n_cores))],
            ins=[cin.opt()], outs=[cout.opt()])
        gs_sb = small.tile([128, 4], F32, tag="gsb", name="gs_sb")
        nc.sync.dma_start(gs_sb[:], cout[:])
        NTOT = float(n_cores * L)
        scale = small.tile([128, 2], F32, tag="scale", name="scale")
        shift = small.tile([128, 2], F32, tag="shift", name="shift")
        mean = small.tile([128, 2], F32, tag="mean", name="mean")
        var = small.tile([128, 2], F32, tag="var", name="var")
        for cc in range(2):
            nc.vector.tensor_scalar(mean[:, cc:cc + 1], gs_sb[:, 2 * cc:2 * cc + 1],
                                    1.0 / NTOT, None, op0=ALU.mult)
            nc.vector.tensor_scalar(var[:, cc:cc + 1], gs_sb[:, 2 * cc + 1:2 * cc + 2],
                                    1.0 / NTOT, None, op0=ALU.mult)
        msq = small.tile([128, 2], F32, tag="msq", name="msq")
        nc.vector.tensor_tensor(msq[:], mean[:], mean[:], op=ALU.mult)
        nc.vector.tensor_tensor(var[:], var[:], msq[:], op=ALU.subtract)
        rs = small.tile([128, 2], F32, tag="rs", name="rs")
        nc.vector.tensor_scalar(var[:], var[:], float(EPS), None, op0=ALU.add)
        nc.scalar.activation(rs[:], var[:], ACT.Sqrt)
        nc.vector.reciprocal(rs[:], rs[:])
        nc.vector.tensor_tensor(scale[:], gamma2[:], rs[:], op=ALU.mult)
        nc.vector.tensor_tensor(shift[:], mean[:], scale[:], op=ALU.mult)
        nc.vector.tensor_tensor(shift[:], beta2[:], shift[:], op=ALU.subtract)

        if DEBUG2:
            nc.gpsimd.dma_start(
                d["dbg_vt"][:].rearrange("p (j c) -> p j c", c=C), vT[:])
            nc.gpsimd.dma_start(d["dbg_wtap"][:], W_tap[:])
            nc.gpsimd.dma_start(
                d["dbg_wtm"][:].rearrange("p (j t) -> p j t", t=25), W_tm[:])
            nc.gpsimd.dma_start(d["dbg_ae"][:], AE[:])
            nc.gpsimd.dma_start(d["dbg_gs"][:], d["gs"][0:NCHUNK * GJ2])
            nc.gpsimd.dma_start(
                d["dbg_out"][:].rearrange("p (m l) -> p m l", l=L), out_all[:])
            for cc in range(2):
                nc.gpsimd.dma_start(d["dbg_xcm"][:, L * cc:L * (cc + 1)],
                                    xT_cm[cc][:])

        # normalize into the dead x1 tiles (bf16), DMA out c-major
        # (host un-transposes and upcasts)
        for n8 in range(8):
            sl = slice(512 * n8, 512 * (n8 + 1))
            for cc in range(2):
                nc.vector.tensor_scalar(x1[cc][:, sl], out_cm[cc][:, sl],
                                        scale[:, cc:cc + 1], shift[:, cc:cc + 1],
                                        op0=ALU.mult, op1=ALU.add)
                eng = nc.sync if (n8 % 2 == 0) else nc.scalar
                eng.dma_start(d["y"][128 * cc:128 * (cc + 1), sl],
                              x1[cc][:, sl])
        cmH.__exit__(None, None, None)


_CACHE = {}


def _get_program(n_cores=N_CORES):
    key = n_cores
    if key not in _CACHE:
        nc = bacc.Bacc("TRN2", target_bir_lowering=False, debug=False,
                       num_devices=n_cores)
        build(nc, n_cores)
        nc.compile()
        _CACHE[key] = nc
    return _CACHE[key]


_CONSTS = None


def make_in_map(inputs, b):
    global _CONSTS
    if _CONSTS is None:
        _CONSTS = host_consts()
    import ml_dtypes
    # pre-permute to the transposed-grid token order l' = w*64 + h
    xbf = np.ascontiguousarray(
        np.asarray(inputs["x"][b]).transpose(1, 0, 2).reshape(L, C)
    ).astype(ml_dtypes.bfloat16)
    return {
        "xbf": xbf,
        "wv": np.ascontiguousarray(inputs["Wv"], np.float32),
        "bv": np.ascontiguousarray(np.asarray(inputs["bv"]).reshape(1, C), np.float32),
        "wa": np.ascontiguousarray(inputs["Wa"], np.float32),
        "ba": np.ascontiguousarray(np.asarray(inputs["ba"]).reshape(81, 1), np.float32),
        "wfu": np.ascontiguousarray(inputs["Wfu"], np.float32),
        "bfu2": np.ascontiguousarray(
            np.asarray(inputs["bfu"]).reshape(2, 128).T, np.float32),
        "gamma2": np.ascontiguousarray(
            np.asarray(inputs["gamma"]).reshape(2, 128).T, np.float32),
        "beta2": np.ascontiguousarray(
            np.asarray(inputs["beta"]).reshape(2, 128).T, np.float32),
        **_CONSTS,
    }


def postprocess(yarr):
    """[256, L] c-major, l' = w*64+h  ->  [H, W, C] in the reference frame."""
    return np.asarray(yarr, np.float32).reshape(C, L).T.reshape(H, W, C)


def kernel(**inputs):
    nc = _get_program()
    in_maps = [make_in_map(inputs, b) for b in range(B)]
    res = run_bass_kernel_spmd(nc, in_maps, list(range(N_CORES)))
    out = np.stack([postprocess(res.results[b]["y"]) for b in range(B)])
    return out.astype(np.float32)



# revision 42
# speedup vs baseline: 1.3339x; 1.3339x over previous
"""Trainium2 Bass kernel for nn_MOA_13254269075617 (sparse windowed attention block).

Sharding: data-parallel over batch B=8 across 8 NeuronCores (1 image each).
BatchNorm uses global batch stats via an on-device AllReduce of per-channel
sum / sum-of-squares.

Per-core pipeline (all in the spatially-TRANSPOSED frame; host pre-permutes
the input to token order l' = w*64 + h and un-permutes the c-major output):
  xT_cm  : x channel-major [2x128, 4096] via transpose-DMAs
  vT     : (x @ Wv + bv) token-major [128, 32, 256] bf16
  A      : softmax(x @ Wa + ba) pq-major [81, 4096] on a zero-padded grid
  W_tap  : 25-tap position-varying stencil weights [25, 4096] (fold+attention
           combined algebraically), wmask'd at grid edges
  G      : banded token->token weight matrices assembled in DRAM by strided
           scatter-DMA (5-tap f-runs = 10B descriptors), stored transposed
           [m, k] and un-transposed by the DMA XBAR on load
  xf     : stencil apply = 6 PE matmuls per 128-token chunk,
           out[c, m] = sum_k vT[k, c] * G[k, m], PSUM-accumulated c-major
  x1/x2  : relu chains with 3x3/5x5 maxpools (separable shifted-max trees)
  out    : concat-matmul (Wfu) + residual, BN with AllReduce'd stats,
           written c-major [256, 4096]; host transposes back
"""
import sys

for _p in (
    "/root/.axon_site",
    "/root/.axon_site/_ro/trn_rl_repo",
    "/root/.axon_site/_ro/pypackages",
    "/opt/trn_rl_repo",
):
    if _p not in sys.path:
        sys.path.append(_p)

from itertools import product

import numpy as np

import concourse.bass as bass
import concourse.tile as tile
from concourse.ap import AP
from concourse import bacc, mybir
from concourse.bass_utils import run_bass_kernel_spmd

F32 = mybir.dt.float32
BF16 = mybir.dt.bfloat16
ALU = mybir.AluOpType
ACT = mybir.ActivationFunctionType

import os
DEBUG2 = os.environ.get("KDEBUG") == "1"
B, H, W, C = 8, 64, 64, 256
L = H * W                      # 4096 tokens
NCHUNK = L // 128              # 32 token chunks
N_CORES = 8
EPS = 1e-5
GROW = 384                     # 3 source blocks x 128 rows per chunk
GJ = GROW * 128                # G elems per chunk
GJ2 = GJ + 128                 # chunk stride in gs: +128 so row images of
                               # adjacent chunks never overlap (no WAR chains)
GSPAD = NCHUNK * GJ2 + GJ + 4096  # guard for conservative OOB checks
DEBUG = False
TAPS = [(e, f) for e in range(-2, 3) for f in range(-2, 3)]


def host_consts():
    """Selector matrices and small constants (host-precomputed, same all cores)."""
    selsum = np.zeros((81, 9), np.float32)
    for p in range(9):
        selsum[9 * p:9 * p + 9, p] = 1.0
    selrep = np.zeros((9, 81), np.float32)
    for p in range(9):
        selrep[p, 9 * p:9 * p + 9] = 1.0
    # selshift[:, 25*d + tap]: for (di,dj) block d, tap (e,f):
    #   k = 9*(3di+dj) + 3(di+e)+(dj+f) if di+e,dj+f in [0,3)
    selshift = np.zeros((81, 9 * 25), np.float32)
    for d, (di, dj) in enumerate(product(range(3), range(3))):
        for t, (e, f) in enumerate(product(range(-2, 3), range(-2, 3))):
            dip, djp = di + e, dj + f
            if 0 <= dip < 3 and 0 <= djp < 3:
                k = 9 * (3 * di + dj) + (3 * dip + djp)
                selshift[k, 25 * d + t] = 1.0
    wmask = np.ones((25, 64, 64), np.float32)
    for t, (e, f) in enumerate(product(range(-2, 3), range(-2, 3))):
        if e > 0: wmask[t, 64 - e:, :] = 0
        if e < 0: wmask[t, :-e, :] = 0
        if f > 0: wmask[t, :, 64 - f:] = 0
        if f < 0: wmask[t, :, :-f] = 0
    import ml_dtypes
    return {
        "selsum": selsum,
        "selrep": selrep,
        "selshift": selshift,
        "wmask": wmask.reshape(25, 4096),
        "ident25": np.eye(25, dtype=np.float32),
        "ident128": np.eye(128, dtype=np.float32),
        "ones1": np.ones((1, 128), np.float32),
        "gs": np.zeros(GSPAD, dtype=ml_dtypes.bfloat16),
    }


def build(nc, n_cores):
    d = {}
    def din(name, shape):
        d[name] = nc.dram_tensor(name, list(shape), F32, kind="ExternalInput").ap()

    d["xbf"] = nc.dram_tensor("xbf", [L, C], BF16, kind="ExternalInput").ap()
    d["gs"] = nc.dram_tensor("gs", [GSPAD], BF16, kind="ExternalInput").ap()
    din("wv", (C, C)); din("bv", (1, C))
    din("wa", (C, 81)); din("ba", (81, 1))
    din("wfu", (2 * C, C)); din("bfu2", (128, 2))
    din("gamma2", (128, 2)); din("beta2", (128, 2))
    din("selsum", (81, 9)); din("selrep", (9, 81)); din("selshift", (81, 225))
    din("ident25", (25, 25)); din("ident128", (128, 128))
    din("ones1", (1, 128)); din("wmask", (25, L))
    d["y"] = nc.dram_tensor("y", [2 * 128, L], BF16, kind="ExternalOutput").ap()
    if DEBUG2:
        d["dbg_vt"] = nc.dram_tensor("dbg_vt", [128, NCHUNK * C], BF16, kind="ExternalOutput").ap()
        d["dbg_wtap"] = nc.dram_tensor("dbg_wtap", [25, L], BF16, kind="ExternalOutput").ap()
        d["dbg_wtm"] = nc.dram_tensor("dbg_wtm", [128, NCHUNK * 25], BF16, kind="ExternalOutput").ap()
        d["dbg_ae"] = nc.dram_tensor("dbg_ae", [81, 66 * 67], BF16, kind="ExternalOutput").ap()
        d["dbg_gs"] = nc.dram_tensor("dbg_gs", [NCHUNK * GJ2], BF16, kind="ExternalOutput").ap()
        d["dbg_out"] = nc.dram_tensor("dbg_out", [128, 2 * L], F32, kind="ExternalOutput").ap()
        d["dbg_xcm"] = nc.dram_tensor("dbg_xcm", [128, 2 * L], BF16, kind="ExternalOutput").ap()
    if DEBUG:
        d["dbg_wtap"] = nc.dram_tensor("dbg_wtap", [25, L], F32, kind="ExternalOutput").ap()
        d["dbg_wtm"] = nc.dram_tensor("dbg_wtm", [128, NCHUNK * 25], F32, kind="ExternalOutput").ap()
        d["dbg_wtmd"] = nc.dram_tensor("dbg_wtmd", [L * 25], BF16, kind="ExternalOutput").ap()
        d["dbg_gs"] = nc.dram_tensor("dbg_gs", [4 * GJ], BF16, kind="ExternalOutput").ap()
        d["dbg_x1"] = nc.dram_tensor("dbg_x1", [2 * 128, L], F32, kind="ExternalOutput").ap()
        d["dbg_vt"] = nc.dram_tensor("dbg_vt", [128, NCHUNK * C], F32, kind="ExternalOutput").ap()

    with tile.TileContext(nc) as tc:
        _build_tc(tc, d, n_cores)
    return d


def _build_tc(tc, d, n_cores):
    nc = tc.nc
    from contextlib import ExitStack
    es = ExitStack()
    with es:
        consts = es.enter_context(tc.tile_pool(name="consts", bufs=1))
        main = es.enter_context(tc.tile_pool(name="main", bufs=1))
        gpool = es.enter_context(tc.tile_pool(name="gpool", bufs=8))
        dram = es.enter_context(tc.tile_pool(name="dram", bufs=2, space="DRAM"))

        # ---- const loads ----
        def cload(name, shape):
            t = consts.tile(list(shape), F32, tag=name, name=name)
            nc.gpsimd.dma_start(t[:], d[name][:])
            return t
        def cload_bf(name, shape):
            t = consts.tile(list(shape), BF16, tag=name, name=name)
            nc.gpsimd.dma_start(t[:], d[name][:])
            return t
        # order by first use: C needs wa/ba, B needs wv/bv/ones1, then
        # softmax/D selectors, then late-phase consts
        wa_sb = consts.tile([128, 2, 81], BF16, tag="wa", name="wa_sb")
        for kc in range(2):
            nc.gpsimd.dma_start(wa_sb[:, kc, :], d["wa"][128 * kc:128 * (kc + 1), :])
        ba_sb = cload("ba", (81, 1))
        wv_sb = consts.tile([128, 2, C], BF16, tag="wv", name="wv_sb")
        for kc in range(2):
            nc.gpsimd.dma_start(wv_sb[:, kc, :], d["wv"][128 * kc:128 * (kc + 1), :])
        ones1 = cload_bf("ones1", (1, 128))
        bv_sb = cload_bf("bv", (1, C))
        selsum = cload_bf("selsum", (81, 9))
        selrep_bf = cload_bf("selrep", (9, 81))
        selshift = cload_bf("selshift", (81, 225))
        ident25 = cload_bf("ident25", (25, 25))
        ident128 = cload_bf("ident128", (128, 128))
        wmask = main.tile([25, L], BF16, tag="wmask", name="wmask")
        nc.gpsimd.dma_start(wmask[:], d["wmask"][:])
        wfu_sb = consts.tile([128, 4, 2, 128], BF16, tag="wfu", name="wfu_sb")
        for kc in range(4):
            for mc in range(2):
                nc.gpsimd.dma_start(
                    wfu_sb[:, kc, mc, :],
                    d["wfu"][128 * kc:128 * (kc + 1), 128 * mc:128 * (mc + 1)])
        bfu2 = cload("bfu2", (128, 2))
        gamma2 = cload("gamma2", (128, 2))
        beta2 = cload("beta2", (128, 2))

        # ---- phase A: straight-load x token-major, PE-transpose to c-major ----
        # (host pre-permuted xbf rows to l' = w*64 + h).  XBAR transpose-DMAs
        # run at ~27GB/s on a single SDMA engine and corrupt each other when
        # two run concurrently; straight DMA + PE transpose is ~5x faster and
        # uses otherwise-idle PE time.
        from concourse.tile import add_dep_helper
        engs = [nc.sync, nc.scalar]
        xT_cm = [main.tile([128, L], BF16, tag=f"xcm{cc}", name=f"xT_cm{cc}")
                 for cc in range(2)]
        cmA = tc.tile_pool(name="psA", bufs=4, space="PSUM"); psA = cmA.__enter__()
        a_trs = {}
        for j in range(NCHUNK):
            xtm = gpool.tile([128, C], BF16, tag="xtm", name="xtm", bufs=6)
            dmx = engs[j % 2].dma_start(xtm[:], d["xbf"][128 * j:128 * (j + 1), :])
            if j - 6 in a_trs:
                add_dep_helper(dmx.ins, a_trs[j - 6].ins, reason="xtm WAR")
            for cc in range(2):
                pa = psA.tile([128, 128], BF16, tag="psa", name="psa")
                tr = nc.tensor.transpose(
                    pa[:], xtm[:, 128 * cc:128 * (cc + 1)], ident128[:])
                if cc == 0:
                    nc.scalar.copy(xT_cm[0][:, 128 * j:128 * (j + 1)], pa[:])
                else:
                    nc.vector.tensor_copy(xT_cm[1][:, 128 * j:128 * (j + 1)],
                                          pa[:])
            a_trs[j] = tr
        cmA.__exit__(None, None, None)

        # ---- phase B: vT = xT @ Wv + bv, token-major (bf16) ----
        cmAB = tc.tile_pool(name="psAB", bufs=3, space="PSUM"); psAB = cmAB.__enter__()
        vT = main.tile([128, NCHUNK, C], BF16, tag="vT", name="vT")
        bv256 = consts.tile([128, C], BF16, tag="bv256", name="bv256")
        psb = psAB.tile([128, C], F32, tag="vps", name="vps")
        nc.tensor.matmul(psb[:], ones1[:], bv_sb[:], start=True, stop=True)
        nc.scalar.copy(bv256[:], psb[:])
        for j in range(NCHUNK):
            ps = psAB.tile([128, C], F32, tag="vps", name="vps")
            nc.tensor.matmul(ps[:], xT_cm[0][:, 128 * j:128 * (j + 1)],
                             wv_sb[:, 0, :], start=True, stop=False)
            nc.tensor.matmul(ps[:], xT_cm[1][:, 128 * j:128 * (j + 1)],
                             wv_sb[:, 1, :], start=False, stop=True)
            nc.vector.tensor_tensor(vT[:, j, :], ps[:], bv256[:], op=ALU.add)

        # ---- phase C: attention logits -> exp -> normalize ----
        cmAB.__exit__(None, None, None)
        cmC = tc.tile_pool(name="psC", bufs=2, space="PSUM"); psC = cmC.__enter__()
        AE = main.tile([81, 66 * 67], BF16, tag="AE", name="AE")
        nc.gpsimd.memset(AE[:], 0.0)
        AE3 = AE.rearrange("p (r s) -> p r s", r=67)

        # NOTE: the automatic dependency tracker is unreliable for the strided
        # AE3 views, so cross-engine RAW edges here are added explicitly
        # (per-engine in-order execution covers the downstream instructions).
        from concourse.tile import add_dep_helper
        exp_insts = []
        for n8 in range(8):
            ps = psC.tile([81, 512], F32, tag="aps", name="aps")
            for kc in range(2):
                nc.tensor.matmul(ps[:], wa_sb[:, kc, :],
                                 xT_cm[kc][:, 512 * n8:512 * (n8 + 1)],
                                 start=(kc == 0), stop=(kc == 1))
            exp_insts.append(nc.scalar.activation(
                AE3[:, 1 + 8 * n8:1 + 8 * n8 + 8, 1:65],
                ps.rearrange("p (r s) -> p r s", s=64),
                ACT.Exp, bias=ba_sb[:, 0:1]))
        ROWCH = [(r0, min(7, 64 - r0)) for r0 in range(0, 64, 7)]
        norm_insts = []
        for r0, nr in ROWCH:
            N = nr * 66
            win = slice((r0 + 1) * 66, (r0 + 1) * 66 + N)
            ps = psC.tile([9, 512], F32, tag="sps", name="sps")
            mm = nc.tensor.matmul(ps[:, 0:N], selsum[:], AE[:, win],
                                  start=True, stop=True)
            # rowsum reads AE rows [r0+1, r0+1+nr): wait for the exp blocks
            for n8 in range(max(0, r0 // 8), min(8, (r0 + nr) // 8 + 1)):
                add_dep_helper(mm.ins, exp_insts[n8].ins,
                               reason="rowsum reads exp'd AE rows")
            rchf = consts.tile([9, 512], F32, tag="rchunkf", name="rchf", bufs=1)
            nc.vector.reciprocal_approx_fast(rchf[:, 0:N], ps[:, 0:N])
            rch = consts.tile([9, 512], BF16, tag="rchunk", name="rch", bufs=1)
            nc.scalar.copy(rch[:, 0:N], rchf[:, 0:N])
            ps2 = psC.tile([81, 512], F32, tag="rps", name="rps")
            nc.tensor.matmul(ps2[:, 0:N], selrep_bf[:], rch[:, 0:N],
                             start=True, stop=True)
            iv = AE3[:, r0 + 1:r0 + 1 + nr, 1:65]
            nv = nc.vector.tensor_tensor(
                iv, iv, ps2[:, 0:N].rearrange("p (r s) -> p r s", s=66)[:, :, 1:65],
                op=ALU.mult)
            for n8 in range(max(0, r0 // 8), min(8, (r0 + nr) // 8 + 1)):
                add_dep_helper(nv.ins, exp_insts[n8].ins,
                               reason="normalize RMWs exp'd AE rows")
            norm_insts.append(nv)

        # ---- phase D: W stencil build (9 shifted selector matmuls) ----
        cmC.__exit__(None, None, None)
        cmD = tc.tile_pool(name="psD", bufs=8, space="PSUM"); psD = cmD.__enter__()
        W_tap = main.tile([25, L], BF16, tag="wtap", name="W_tap")
        wmask_t = wmask.rearrange("p (u v) -> p v u", u=64)
        wtap_t = W_tap.rearrange("p (u v) -> p v u", u=64)
        ev_insts = []
        first_d = True
        for r0, nr in ROWCH:
            N = nr * 66
            ps = psD.tile([25, 512], F32, tag="wps", name="wps")
            for dd, (di, dj) in enumerate(product(range(3), range(3))):
                st = (r0 + 2 - dj) * 66 + (2 - di)
                mm = nc.tensor.matmul(ps[:, 0:N],
                                      selshift[:, 25 * dd:25 * (dd + 1)],
                                      AE[:, st:st + N],
                                      start=(dd == 0), stop=(dd == 8))
                if first_d:
                    # PE is in-order: gating the first D matmul on all
                    # normalizes covers every later AE read in phase D
                    for nv in norm_insts:
                        add_dep_helper(mm.ins, nv.ins,
                                       reason="D reads normalized AE")
                    first_d = False
            ev_insts.append(nc.vector.tensor_tensor(
                wtap_t[:, r0:r0 + nr, :],
                ps[:, 0:N].rearrange("p (r s) -> p r s", s=66)[:, :, 0:64],
                wmask_t[:, r0:r0 + nr, :], op=ALU.mult))
        cmD.__exit__(None, None, None)

        # ---- phase D2: transpose W to token-major (SBUF only) ----
        cmD2 = tc.tile_pool(name="psD2", bufs=3, space="PSUM"); psD2 = cmD2.__enter__()
        W_tm = main.tile([128, NCHUNK, 25], BF16, tag="wtm", name="W_tm")
        if DEBUG:
            nc.gpsimd.dma_start(d["dbg_wtap"][:], W_tap[:])
        # LDWEIGHTS prefetch hazard: tr's stationary is W_tap, whose writes go
        # through a strided view the tracker misses, and a dep ON tr gates the
        # matmul entry, not its LDW (which executes before the wait).  Gate a
        # PE NOP queue entry on the evacs so no LDW can read W_tap early.
        pe_guard = nc.tensor.nop(nofuse=True, hint="D2 W_tap guard")
        for ev in ev_insts:
            add_dep_helper(pe_guard.ins, ev.ins, reason="D2 reads W_tap")
        wtm_copies = []
        for j in range(NCHUNK):
            pt = psD2.tile([128, 25], BF16, tag="wtp", name="wtp")
            tr = nc.tensor.transpose(pt[:], W_tap[:, 128 * j:128 * (j + 1)],
                                     ident25[:])
            if j == 0:
                add_dep_helper(tr.ins, pe_guard.ins, reason="D2 after guard")
            wtm_copies.append(nc.scalar.copy(W_tm[:, j, :], pt[:]))
        cmD2.__exit__(None, None, None)
        # ---- G^T row-image build, one plain strided store per chunk ----
        # GT[j][m, k] = weight linking source token 128*(j + k//128 - 1) + k%128
        # to output token 128*j + m; tap (e,f) occupies k = m + 64e + f + 128,
        # i.e. flat position 385*m + 64e + f + 128 (+2 global shift so row
        # windows are non-negative).  Row m's 25 taps live at in-row offsets
        # 64*(e+2) + (f+2) of a [128, 384] SBUF image whose gaps are zero, so
        # one strided SBUF copy + one 2D-strided DRAM store per chunk replaces
        # the diagonal scatter.  Geometric clipping (k outside [0, 384)) is
        # exactly the set of taps wmask already zeroed, so no edge cases.
        # The store dst/load src are plain 2D APs the dependency tracker can
        # range-analyze, giving completion-accurate store->load edges.
        gs_t = d["gs"].tensor
        engs = [nc.sync, nc.scalar]
        probe_t = consts.tile([1, 8], F32, tag="probe", name="probe")

        # ---- phase F: maxpools on xT_cm (channel-major grid) ----
        ptmp = es.enter_context(tc.tile_pool(name="ptmp", bufs=3))
        mp_copies = []
        m1 = [main.tile([128, L], BF16, tag=f"m1{cc}", name=f"m1_{cc}") for cc in range(2)]
        m2 = [main.tile([128, L], BF16, tag=f"m2{cc}", name=f"m2_{cc}") for cc in range(2)]

        def g3(ap):
            return ap.rearrange("p (h w) -> p h w", h=64)

        def hmax3(eng, dst, src):
            dv, sv = g3(dst), g3(src)
            t1 = ptmp.tile([128, L], BF16, tag="ptmp", name="ptmp")
            tv = g3(t1)
            eng.tensor_tensor(tv[:, :, 1:], sv[:, :, 1:], sv[:, :, :63], op=ALU.max)
            mp_copies.append(nc.scalar.copy(tv[:, :, 0:1], sv[:, :, 0:1]))
            eng.tensor_tensor(dv[:, :, :63], tv[:, :, :63], sv[:, :, 1:], op=ALU.max)
            mp_copies.append(nc.scalar.copy(dv[:, :, 63:64], tv[:, :, 63:64]))

        def vmax3(eng, dst, src):
            dv, sv = g3(dst), g3(src)
            t1 = ptmp.tile([128, L], BF16, tag="ptmp", name="ptmp")
            tv = g3(t1)
            eng.tensor_tensor(tv[:, 1:, :], sv[:, 1:, :], sv[:, :63, :], op=ALU.max)
            mp_copies.append(nc.scalar.copy(tv[:, 0:1, :], sv[:, 0:1, :]))
            eng.tensor_tensor(dv[:, :63, :], tv[:, :63, :], sv[:, 1:, :], op=ALU.max)
            mp_copies.append(nc.scalar.copy(dv[:, 63:64, :], tv[:, 63:64, :]))

        def hspread(eng, dst, src):   # dst[v] = max(src[v-1], src[v+1]) + edge copies
            dv, sv = g3(dst), g3(src)
            eng.tensor_tensor(dv[:, :, 1:63], sv[:, :, 0:62], sv[:, :, 2:64], op=ALU.max)
            mp_copies.append(nc.scalar.copy(dv[:, :, 0:1], sv[:, :, 1:2]))
            mp_copies.append(nc.scalar.copy(dv[:, :, 63:64], sv[:, :, 62:63]))

        def vspread(eng, dst, src):
            dv, sv = g3(dst), g3(src)
            eng.tensor_tensor(dv[:, 1:63, :], sv[:, 0:62, :], sv[:, 2:64, :], op=ALU.max)
            mp_copies.append(nc.scalar.copy(dv[:, 0:1, :], sv[:, 1:2, :]))
            mp_copies.append(nc.scalar.copy(dv[:, 63:64, :], sv[:, 62:63, :]))

        for cc in range(2):
            eng = nc.vector
            cm3 = ptmp.tile([128, L], BF16, tag="ptmp", name="ptmp")
            hmax3(eng, cm3, xT_cm[cc])
            vmax3(eng, m1[cc], cm3)
            cm5 = ptmp.tile([128, L], BF16, tag="ptmp", name="ptmp")
            hspread(eng, cm5, cm3)
            r35 = ptmp.tile([128, L], BF16, tag="ptmp", name="ptmp")
            vmax3(eng, r35, cm5)
            vspread(eng, m2[cc], r35)

        # ---- phase E: banded stencil apply, c-major out ----
        cmE = tc.tile_pool(name="psE", bufs=2, space="PSUM"); psE = cmE.__enter__()
        x1 = [main.tile([128, L], BF16, tag=f"x1{cc}", name=f"x1_{cc}") for cc in range(2)]
        x2 = [main.tile([128, L], BF16, tag=f"x2{cc}", name=f"x2_{cc}") for cc in range(2)]
        # ONE XBAR load per chunk: G rows (m, b) interleave contiguously at
        # stride 128 (= the XBAR column count), i.e. the transpose input is a
        # plain contiguous [384, 128] block. Column m of source block b then
        # sits at rhs position 3*m + b (stride-3 moving-operand AP).
        last_mm_of_chunk = {}
        gst_of_chunk = {}
        gtr_of_chunk = {}
        for j in range(NCHUNK):
            # build the [128, 384] zero-gapped row image for chunk j and
            # store it as one 2D-strided DMA (row m -> gs[j*GJ + 385*m ..])
            gsb = gpool.tile([128, 384], BF16, tag="gsb", name="gsb", bufs=4)
            if j < 4:
                nc.vector.memset(gsb[:], 0.0)
            cp = nc.vector.tensor_scalar(
                gsb.rearrange("p (g c) -> p g c", c=64)[:, 0:5, 0:5],
                W_tm[:, j, :].rearrange("p (g f) -> p g f", f=5),
                1.0, None, op0=ALU.mult)
            add_dep_helper(cp.ins, wtm_copies[j].ins, reason="gsb reads W_tm")
            if j - 4 in gst_of_chunk:
                add_dep_helper(cp.ins, gst_of_chunk[j - 4].ins,
                               reason="gsb buffer WAR vs store")
            st = nc.scalar.dma_start(
                AP(tensor=gs_t, offset=j * GJ2, ap=[[385, 128], [1, 261]]),
                gsb[:, 0:261])
            add_dep_helper(st.ins, cp.ins, reason="store reads gsb")
            gst_of_chunk[j] = st
            # straight-load the [384, 128] row image as 3 partition blocks and
            # un-transpose on the PE (XBAR transpose-DMA is 5x slower and
            # corrupts under concurrency)
            graw = gpool.tile([128, 3, 128], BF16, tag="graw", name="graw",
                              bufs=4)
            ld = nc.sync.dma_start(
                graw[:],
                AP(tensor=gs_t, offset=j * GJ2 + 2,
                   ap=[[128, 128], [16384, 3], [1, 128]]))
            add_dep_helper(ld.ins, st.ins, reason="G load after store")
            if j - 4 in gtr_of_chunk:
                add_dep_helper(ld.ins, gtr_of_chunk[j - 4].ins,
                               reason="graw buffer WAR")
            g = gpool.tile([128, 384], BF16, tag="g", name="g")
            g3v = g.rearrange("k (m b) -> k m b", b=3)
            gnop = nc.tensor.nop(nofuse=True, hint="G LDW guard")
            add_dep_helper(gnop.ins, ld.ins, reason="G transpose reads load")
            for b2 in range(3):
                pgt = psE.tile([128, 128], BF16, tag="gtr", name="gtr", bufs=3)
                gtr = nc.tensor.transpose(pgt[:], graw[:, b2, :], ident128[:])
                if b2 == 0:
                    add_dep_helper(gtr.ins, gnop.ins, reason="after LDW guard")
                gcp = nc.scalar.copy(g[:, 128 * b2:128 * (b2 + 1)], pgt[:])
                if b2 == 0 and j - 8 in last_mm_of_chunk:
                    add_dep_helper(gcp.ins, last_mm_of_chunk[j - 8].ins,
                                   reason="g-buffer WAR")
            gtr_of_chunk[j] = gtr
            bs = [b for b in range(3) if 0 <= j + b - 1 < NCHUNK]
            first_of_chunk = True
            for cc in range(2):
                psx = psE.tile([128, 128], F32, tag=f"psx{cc}", name="psx")
                for i, b in enumerate(bs):
                    mm = nc.tensor.matmul(
                        psx[:],
                        vT[:, j + b - 1, 128 * cc:128 * (cc + 1)],
                        g3v[:, :, b],
                        start=(i == 0), stop=(i == len(bs) - 1))
                    if first_of_chunk:
                        # matmuls read g through the stride-3 view; explicit
                        # edge on the scalar evac (PE in-order covers rest)
                        add_dep_helper(mm.ins, gcp.ins,
                                       reason="E matmul reads g evacs")
                        first_of_chunk = False
                last_mm_of_chunk[j] = mm
                nc.scalar.activation(x1[cc][:, 128 * j:128 * (j + 1)],
                                     psx[:], ACT.Relu)
        cmE.__exit__(None, None, None)

        if DEBUG:
            for cc in range(2):
                nc.gpsimd.dma_start(d["dbg_x1"][128 * cc:128 * (cc + 1), :], x1[cc][:])
            nc.gpsimd.dma_start(
                d["dbg_vt"][:].rearrange("p (j c) -> p j c", c=C), vT[:])

        # ---- phase G tail: x1 = relu(xr + m1); x2 = relu(x1 + m2) ----
        first_gt = True
        for n8 in range(8):
            sl = slice(512 * n8, 512 * (n8 + 1))
            for cc in range(2):
                gt = nc.vector.tensor_tensor(x1[cc][:, sl], x1[cc][:, sl],
                                             m1[cc][:, sl], op=ALU.add)
                if first_gt:
                    for cp in mp_copies:
                        add_dep_helper(gt.ins, cp.ins,
                                       reason="m-add reads maxpool edge fills")
                    first_gt = False
                nc.scalar.activation(x1[cc][:, sl], x1[cc][:, sl], ACT.Relu)
                nc.vector.tensor_tensor(x2[cc][:, sl], x1[cc][:, sl],
                                        m2[cc][:, sl], op=ALU.add)
                nc.scalar.activation(x2[cc][:, sl], x2[cc][:, sl], ACT.Relu)

        # ---- phase H: fu matmul + residual (mc-outer), BN per half ----
        cmH = tc.tile_pool(name="psH", bufs=4, space="PSUM"); psH = cmH.__enter__()
        out_all = main.tile([128, 2, L], F32, tag="out", name="out_all")
        out_cm = [out_all[:, cc, :] for cc in range(2)]
        small = es.enter_context(tc.tile_pool(name="small", bufs=1))
        bnpack = small.tile([128, 4], F32, tag="bnpack", name="bnpack")
        cin = dram.tile([128, 4], F32, name="cin")
        cout = dram.tile([128, 4], F32, name="cout")
        rhss = [x1[0], x1[1], x2[0], x2[1]]
        for mc in range(2):
            for n8 in range(8):
                sl = slice(512 * n8, 512 * (n8 + 1))
                ps = psH.tile([128, 512], F32, tag="fups", name="fups")
                for kc in range(4):
                    nc.tensor.matmul(ps[:], wfu_sb[:, kc, mc, :],
                                     rhss[kc][:, sl],
                                     start=(kc == 0), stop=(kc == 3))
                nc.scalar.activation(out_cm[mc][:, sl], ps[:], ACT.Relu,
                                     bias=bfu2[:, mc:mc + 1])
                nc.vector.tensor_tensor(out_cm[mc][:, sl], out_cm[mc][:, sl],
                                        xT_cm[mc][:, sl], op=ALU.add)
            st = small.tile([128, 8, 6], F32, tag="bnst", name="bnst")
            for n8 in range(8):
                nc.vector.bn_stats(st[:, n8, :], out_cm[mc][:, 512 * n8:512 * (n8 + 1)])
            ag = small.tile([128, 2], F32, tag="bnag", name="bnag")
            nc.vector.bn_aggr(ag[:], st[:])
            nc.vector.tensor_scalar(bnpack[:, 2 * mc:2 * mc + 1], ag[:, 0:1],
                                    float(L), None, op0=ALU.mult)
            sq = small.tile([128, 1], F32, tag="bnsq", name="bnsq")
            nc.vector.tensor_tensor(sq[:], ag[:, 0:1], ag[:, 0:1], op=ALU.mult)
            nc.vector.tensor_tensor(sq[:], sq[:], ag[:, 1:2], op=ALU.add)
            nc.vector.tensor_scalar(bnpack[:, 2 * mc + 1:2 * mc + 2], sq[:],
                                    float(L), None, op0=ALU.mult)
        # one packed AllReduce for both halves' (sum, sumsq)
        nc.sync.dma_start(cin[:], bnpack[:])
        nc.gpsimd.collective_compute(
            "AllReduce", ALU.add,
            replica_groups=[list(range(n_cores))],
            ins=[cin.opt()], outs=[cout.opt()])
        gs_sb = small.tile([128, 4], F32, tag="gsb", name="gs_sb")
        nc.sync.dma_start(gs_sb[:], cout[:])
        NTOT = float(n_cores * L)
        scale = small.tile([128, 2], F32, tag="scale", name="scale")
        shift = small.tile([128, 2], F32, tag="shift", name="shift")
        mean = small.tile([128, 2], F32, tag="mean", name="mean")
        var = small.tile([128, 2], F32, tag="var", name="var")
        for cc in range(2):
            nc.vector.tensor_scalar(mean[:, cc:cc + 1], gs_sb[:, 2 * cc:2 * cc + 1],
                                    1.0 / NTOT, None, op0=ALU.mult)
            nc.vector.tensor_scalar(var[:, cc:cc + 1], gs_sb[:, 2 * cc + 1:2 * cc + 2],
                                    1.0 / NTOT, None, op0=ALU.mult)
        msq = small.tile([128, 2], F32, tag="msq", name="msq")
        nc.vector.tensor_tensor(msq[:], mean[:], mean[:], op=ALU.mult)
        nc.vector.tensor_tensor(var[:], var[:], msq[:], op=ALU.subtract)
        rs = small.tile([128, 2], F32, tag="rs", name="rs")
        nc.vector.tensor_scalar(var[:], var[:], float(EPS), None, op0=ALU.add)
        nc.scalar.activation(rs[:], var[:], ACT.Sqrt)
        nc.vector.reciprocal(rs[:], rs[:])
        nc.vector.tensor_tensor(scale[:], gamma2[:], rs[:], op=ALU.mult)
        nc.vector.tensor_tensor(shift[:], mean[:], scale[:], op=ALU.mult)
        nc.vector.tensor_tensor(shift[:], beta2[:], shift[:], op=ALU.subtract)

        if DEBUG2:
            nc.gpsimd.dma_start(
                d["dbg_vt"][:].rearrange("p (j c) -> p j c", c=C), vT[:])
            nc.gpsimd.dma_start(d["dbg_wtap"][:], W_tap[:])
            nc.gpsimd.dma_start(
                d["dbg_wtm"][:].rearrange("p (j t) -> p j t", t=25), W_tm[:])
            nc.gpsimd.dma_start(d["dbg_ae"][:], AE[:])
            nc.gpsimd.dma_start(d["dbg_gs"][:], d["gs"][0:NCHUNK * GJ2])
            nc.gpsimd.dma_start(
                d["dbg_out"][:].rearrange("p (m l) -> p m l", l=L), out_all[:])
            for cc in range(2):
                nc.gpsimd.dma_start(d["dbg_xcm"][:, L * cc:L * (cc + 1)],
                                    xT_cm[cc][:])

        # normalize into the dead x1 tiles (bf16), DMA out c-major
        # (host un-transposes and upcasts)
        for n8 in range(8):
            sl = slice(512 * n8, 512 * (n8 + 1))
            for cc in range(2):
                nc.vector.tensor_scalar(x1[cc][:, sl], out_cm[cc][:, sl],
                                        scale[:, cc:cc + 1], shift[:, cc:cc + 1],
                                        op0=ALU.mult, op1=ALU.add)
                eng = nc.sync if (n8 % 2 == 0) else nc.scalar
                eng.dma_start(d["y"][128 * cc:128 * (cc + 1), sl],
                              x1[cc][:, sl])
        cmH.__exit__(None, None, None)


_CACHE = {}


def _get_program(n_cores=N_CORES):
    key = n_cores
    if key not in _CACHE:
        nc = bacc.Bacc("TRN2", target_bir_lowering=False, debug=False,
                       num_devices=n_cores)
        build(nc, n_cores)
        nc.compile()
        _CACHE[key] = nc
    return _CACHE[key]


_CONSTS = None


def make_in_map(inputs, b):
    global _CONSTS
    if _CONSTS is None:
        _CONSTS = host_consts()
    import ml_dtypes
    # pre-permute to the transposed-grid token order l' = w*64 + h
    xbf = np.ascontiguousarray(
        np.asarray(inputs["x"][b]).transpose(1, 0, 2).reshape(L, C)
    ).astype(ml_dtypes.bfloat16)
    return {
        "xbf": xbf,
        "wv": np.ascontiguousarray(inputs["Wv"], np.float32),
        "bv": np.ascontiguousarray(np.asarray(inputs["bv"]).reshape(1, C), np.float32),
        "wa": np.ascontiguousarray(inputs["Wa"], np.float32),
        "ba": np.ascontiguousarray(np.asarray(inputs["ba"]).reshape(81, 1), np.float32),
        "wfu": np.ascontiguousarray(inputs["Wfu"], np.float32),
        "bfu2": np.ascontiguousarray(
            np.asarray(inputs["bfu"]).reshape(2, 128).T, np.float32),
        "gamma2": np.ascontiguousarray(
            np.asarray(inputs["gamma"]).reshape(2, 128).T, np.float32),
        "beta2": np.ascontiguousarray(
            np.asarray(inputs["beta"]).reshape(2, 128).T, np.float32),
        **_CONSTS,
    }


def postprocess(yarr):
    """[256, L] c-major, l' = w*64+h  ->  [H, W, C] in the reference frame."""
    return np.asarray(yarr, np.float32).reshape(C, L).T.reshape(H, W, C)


def kernel(**inputs):
    nc = _get_program()
    in_maps = [make_in_map(inputs, b) for b in range(B)]
    res = run_bass_kernel_spmd(nc, in_maps, list(range(N_CORES)))
    out = np.stack([postprocess(res.results[b]["y"]) for b in range(B)])
    return out.astype(np.float32)



# revision 49
# speedup vs baseline: 1.5125x; 1.1338x over previous
"""Trainium2 Bass kernel for nn_MOA_13254269075617 (sparse windowed attention block).

Sharding: data-parallel over batch B=8 across 8 NeuronCores (1 image each).
BatchNorm uses global batch stats via an on-device AllReduce of per-channel
sum / sum-of-squares.

Per-core pipeline (all in the spatially-TRANSPOSED frame; host pre-permutes
the input to token order l' = w*64 + h and un-permutes the c-major output):
  xT_cm  : x channel-major [2x128, 4096] via transpose-DMAs
  vT     : (x @ Wv + bv) token-major [128, 32, 256] bf16
  A      : softmax(x @ Wa + ba) pq-major [81, 4096] on a zero-padded grid
  W_tap  : 25-tap position-varying stencil weights [25, 4096] (fold+attention
           combined algebraically), wmask'd at grid edges
  G      : banded token->token weight matrices assembled in DRAM by strided
           scatter-DMA (5-tap f-runs = 10B descriptors), stored transposed
           [m, k] and un-transposed by the DMA XBAR on load
  xf     : stencil apply = 6 PE matmuls per 128-token chunk,
           out[c, m] = sum_k vT[k, c] * G[k, m], PSUM-accumulated c-major
  x1/x2  : relu chains with 3x3/5x5 maxpools (separable shifted-max trees)
  out    : concat-matmul (Wfu) + residual, BN with AllReduce'd stats,
           written c-major [256, 4096]; host transposes back
"""
import sys

for _p in (
    "/root/.axon_site",
    "/root/.axon_site/_ro/trn_rl_repo",
    "/root/.axon_site/_ro/pypackages",
    "/opt/trn_rl_repo",
):
    if _p not in sys.path:
        sys.path.append(_p)

from itertools import product

import numpy as np

import concourse.bass as bass
import concourse.tile as tile
from concourse.ap import AP
from concourse import bacc, mybir
from concourse.bass_utils import run_bass_kernel_spmd

F32 = mybir.dt.float32
BF16 = mybir.dt.bfloat16
ALU = mybir.AluOpType
ACT = mybir.ActivationFunctionType

import os
DEBUG2 = os.environ.get("KDEBUG") == "1"
B, H, W, C = 8, 64, 64, 256
L = H * W                      # 4096 tokens
NCHUNK = L // 128              # 32 token chunks
N_CORES = 8
EPS = 1e-5
GROW = 384                     # 3 source blocks x 128 rows per chunk
GJ = GROW * 128                # G elems per chunk
GJ2 = GJ + 128                 # chunk stride in gs: +128 so row images of
                               # adjacent chunks never overlap (no WAR chains)
GSPAD = NCHUNK * GJ2 + GJ + 4096  # guard for conservative OOB checks
DEBUG = False
TAPS = [(e, f) for e in range(-2, 3) for f in range(-2, 3)]


def host_consts():
    """Selector matrices and small constants (host-precomputed, same all cores)."""
    selsum = np.zeros((81, 9), np.float32)
    for p in range(9):
        selsum[9 * p:9 * p + 9, p] = 1.0
    selrep = np.zeros((9, 81), np.float32)
    for p in range(9):
        selrep[p, 9 * p:9 * p + 9] = 1.0
    # selshift[:, 25*d + tap]: for (di,dj) block d, tap (e,f):
    #   k = 9*(3di+dj) + 3(di+e)+(dj+f) if di+e,dj+f in [0,3)
    selshift = np.zeros((81, 9 * 25), np.float32)
    for d, (di, dj) in enumerate(product(range(3), range(3))):
        for t, (e, f) in enumerate(product(range(-2, 3), range(-2, 3))):
            dip, djp = di + e, dj + f
            if 0 <= dip < 3 and 0 <= djp < 3:
                k = 9 * (3 * di + dj) + (3 * dip + djp)
                selshift[k, 25 * d + t] = 1.0
    wmask = np.ones((25, 64, 64), np.float32)
    for t, (e, f) in enumerate(product(range(-2, 3), range(-2, 3))):
        if e > 0: wmask[t, 64 - e:, :] = 0
        if e < 0: wmask[t, :-e, :] = 0
        if f > 0: wmask[t, :, 64 - f:] = 0
        if f < 0: wmask[t, :, :-f] = 0
    import ml_dtypes
    return {
        "selsum": selsum,
        "selrep": selrep,
        "selshift": selshift,
        "wmask": wmask.reshape(25, 4096),
        "ident25": np.eye(25, dtype=np.float32),
        "ident128": np.eye(128, dtype=np.float32),
        "ones1": np.ones((1, 128), np.float32),
        "gs": np.zeros(GSPAD, dtype=ml_dtypes.bfloat16),
    }


def build(nc, n_cores):
    d = {}
    def din(name, shape):
        d[name] = nc.dram_tensor(name, list(shape), F32, kind="ExternalInput").ap()

    d["xbf"] = nc.dram_tensor("xbf", [L, C], BF16, kind="ExternalInput").ap()
    d["gs"] = nc.dram_tensor("gs", [GSPAD], BF16, kind="ExternalInput").ap()
    din("wv", (C, C)); din("bv", (1, C))
    din("wa", (C, 81)); din("ba", (81, 1))
    din("wfu", (2 * C, C)); din("bfu2", (128, 2))
    din("gamma2", (128, 2)); din("beta2", (128, 2))
    din("selsum", (81, 9)); din("selrep", (9, 81)); din("selshift", (81, 225))
    din("ident25", (25, 25)); din("ident128", (128, 128))
    din("ones1", (1, 128)); din("wmask", (25, L))
    d["y"] = nc.dram_tensor("y", [2 * 128, L], BF16, kind="ExternalOutput").ap()
    if DEBUG2:
        d["dbg_vt"] = nc.dram_tensor("dbg_vt", [128, NCHUNK * C], BF16, kind="ExternalOutput").ap()
        d["dbg_wtap"] = nc.dram_tensor("dbg_wtap", [25, L], BF16, kind="ExternalOutput").ap()
        d["dbg_wtm"] = nc.dram_tensor("dbg_wtm", [128, NCHUNK * 25], BF16, kind="ExternalOutput").ap()
        d["dbg_ae"] = nc.dram_tensor("dbg_ae", [81, 66 * 67], BF16, kind="ExternalOutput").ap()
        d["dbg_gs"] = nc.dram_tensor("dbg_gs", [NCHUNK * GJ2], BF16, kind="ExternalOutput").ap()
        d["dbg_out"] = nc.dram_tensor("dbg_out", [128, 2 * L], F32, kind="ExternalOutput").ap()
        d["dbg_xcm"] = nc.dram_tensor("dbg_xcm", [128, 2 * L], BF16, kind="ExternalOutput").ap()
    if DEBUG:
        d["dbg_wtap"] = nc.dram_tensor("dbg_wtap", [25, L], F32, kind="ExternalOutput").ap()
        d["dbg_wtm"] = nc.dram_tensor("dbg_wtm", [128, NCHUNK * 25], F32, kind="ExternalOutput").ap()
        d["dbg_wtmd"] = nc.dram_tensor("dbg_wtmd", [L * 25], BF16, kind="ExternalOutput").ap()
        d["dbg_gs"] = nc.dram_tensor("dbg_gs", [4 * GJ], BF16, kind="ExternalOutput").ap()
        d["dbg_x1"] = nc.dram_tensor("dbg_x1", [2 * 128, L], F32, kind="ExternalOutput").ap()
        d["dbg_vt"] = nc.dram_tensor("dbg_vt", [128, NCHUNK * C], F32, kind="ExternalOutput").ap()

    with tile.TileContext(nc) as tc:
        _build_tc(tc, d, n_cores)
    return d


def _build_tc(tc, d, n_cores):
    nc = tc.nc
    from contextlib import ExitStack
    es = ExitStack()
    with es:
        consts = es.enter_context(tc.tile_pool(name="consts", bufs=1))
        main = es.enter_context(tc.tile_pool(name="main", bufs=1))
        gpool = es.enter_context(tc.tile_pool(name="gpool", bufs=8))
        dram = es.enter_context(tc.tile_pool(name="dram", bufs=2, space="DRAM"))

        # ---- const loads ----
        def cload(name, shape):
            t = consts.tile(list(shape), F32, tag=name, name=name)
            nc.gpsimd.dma_start(t[:], d[name][:])
            return t
        def cload_bf(name, shape):
            t = consts.tile(list(shape), BF16, tag=name, name=name)
            nc.gpsimd.dma_start(t[:], d[name][:])
            return t
        # order by first use: phase A needs ident128, B needs ones1/bv/wv,
        # C needs wa/ba, then selectors, then late-phase consts
        ident128 = cload_bf("ident128", (128, 128))
        ones1 = cload_bf("ones1", (1, 128))
        bv_sb = cload_bf("bv", (1, C))
        wv_sb = consts.tile([128, 2, C], BF16, tag="wv", name="wv_sb")
        for kc in range(2):
            nc.gpsimd.dma_start(wv_sb[:, kc, :], d["wv"][128 * kc:128 * (kc + 1), :])
        wa_sb = consts.tile([128, 2, 81], BF16, tag="wa", name="wa_sb")
        for kc in range(2):
            nc.gpsimd.dma_start(wa_sb[:, kc, :], d["wa"][128 * kc:128 * (kc + 1), :])
        ba_sb = cload("ba", (81, 1))
        selsum = cload_bf("selsum", (81, 9))
        selrep_bf = cload_bf("selrep", (9, 81))
        selshift = cload_bf("selshift", (81, 225))
        ident25 = cload_bf("ident25", (25, 25))
        wmask = main.tile([25, L], BF16, tag="wmask", name="wmask")
        nc.gpsimd.dma_start(wmask[:], d["wmask"][:])
        wfu_sb = consts.tile([128, 4, 2, 128], BF16, tag="wfu", name="wfu_sb")
        for kc in range(4):
            for mc in range(2):
                nc.gpsimd.dma_start(
                    wfu_sb[:, kc, mc, :],
                    d["wfu"][128 * kc:128 * (kc + 1), 128 * mc:128 * (mc + 1)])
        bfu2 = cload("bfu2", (128, 2))
        gamma2 = cload("gamma2", (128, 2))
        beta2 = cload("beta2", (128, 2))

        # ---- phase A: straight-load x token-major, PE-transpose to c-major ----
        # (host pre-permuted xbf rows to l' = w*64 + h).  XBAR transpose-DMAs
        # run at ~27GB/s on a single SDMA engine and corrupt each other when
        # two run concurrently; straight DMA + PE transpose is ~5x faster and
        # uses otherwise-idle PE time.
        from concourse.tile import add_dep_helper
        engs = [nc.sync, nc.scalar]
        xT_cm = [main.tile([128, L], BF16, tag=f"xcm{cc}", name=f"xT_cm{cc}")
                 for cc in range(2)]
        cmA = tc.tile_pool(name="psA", bufs=4, space="PSUM"); psA = cmA.__enter__()
        a_trs = {}
        for j in range(NCHUNK):
            xtm = gpool.tile([128, C], BF16, tag="xtm", name="xtm", bufs=6)
            dmx = engs[j % 2].dma_start(xtm[:], d["xbf"][128 * j:128 * (j + 1), :])
            if j - 6 in a_trs:
                add_dep_helper(dmx.ins, a_trs[j - 6].ins, reason="xtm WAR")
            for cc in range(2):
                pa = psA.tile([128, 128], BF16, tag="psa", name="psa")
                tr = nc.tensor.transpose(
                    pa[:], xtm[:, 128 * cc:128 * (cc + 1)], ident128[:])
                if cc == 0:
                    nc.scalar.copy(xT_cm[0][:, 128 * j:128 * (j + 1)], pa[:])
                else:
                    nc.vector.tensor_copy(xT_cm[1][:, 128 * j:128 * (j + 1)],
                                          pa[:])
            a_trs[j] = tr
        cmA.__exit__(None, None, None)

        # ---- phase B: vT = xT @ Wv + bv, token-major (bf16) ----
        cmAB = tc.tile_pool(name="psAB", bufs=3, space="PSUM"); psAB = cmAB.__enter__()
        vT = main.tile([128, NCHUNK, C], BF16, tag="vT", name="vT")
        bv256 = consts.tile([128, C], BF16, tag="bv256", name="bv256")
        psb = psAB.tile([128, C], F32, tag="vps", name="vps")
        nc.tensor.matmul(psb[:], ones1[:], bv_sb[:], start=True, stop=True)
        nc.scalar.copy(bv256[:], psb[:])
        for j in range(NCHUNK):
            ps = psAB.tile([128, C], F32, tag="vps", name="vps")
            nc.tensor.matmul(ps[:], xT_cm[0][:, 128 * j:128 * (j + 1)],
                             wv_sb[:, 0, :], start=True, stop=False)
            nc.tensor.matmul(ps[:], xT_cm[1][:, 128 * j:128 * (j + 1)],
                             wv_sb[:, 1, :], start=False, stop=True)
            nc.vector.tensor_tensor(vT[:, j, :], ps[:], bv256[:], op=ALU.add)

        # ---- phase C: attention logits -> exp -> normalize ----
        cmAB.__exit__(None, None, None)
        cmC = tc.tile_pool(name="psC", bufs=2, space="PSUM"); psC = cmC.__enter__()
        AE = main.tile([81, 66 * 67], BF16, tag="AE", name="AE")
        nc.gpsimd.memset(AE[:], 0.0)
        AE3 = AE.rearrange("p (r s) -> p r s", r=67)

        # NOTE: the automatic dependency tracker is unreliable for the strided
        # AE3 views, so cross-engine RAW edges here are added explicitly
        # (per-engine in-order execution covers the downstream instructions).
        from concourse.tile import add_dep_helper
        exp_insts = []
        for n8 in range(8):
            ps = psC.tile([81, 512], F32, tag="aps", name="aps")
            for kc in range(2):
                nc.tensor.matmul(ps[:], wa_sb[:, kc, :],
                                 xT_cm[kc][:, 512 * n8:512 * (n8 + 1)],
                                 start=(kc == 0), stop=(kc == 1))
            exp_insts.append(nc.scalar.activation(
                AE3[:, 1 + 8 * n8:1 + 8 * n8 + 8, 1:65],
                ps.rearrange("p (r s) -> p r s", s=64),
                ACT.Exp, bias=ba_sb[:, 0:1]))
        ROWCH = [(r0, min(7, 64 - r0)) for r0 in range(0, 64, 7)]
        norm_insts = []
        for r0, nr in ROWCH:
            N = nr * 66
            win = slice((r0 + 1) * 66, (r0 + 1) * 66 + N)
            ps = psC.tile([9, 512], F32, tag="sps", name="sps")
            mm = nc.tensor.matmul(ps[:, 0:N], selsum[:], AE[:, win],
                                  start=True, stop=True)
            # rowsum reads AE rows [r0+1, r0+1+nr): wait for the exp blocks
            for n8 in range(max(0, r0 // 8), min(8, (r0 + nr) // 8 + 1)):
                add_dep_helper(mm.ins, exp_insts[n8].ins,
                               reason="rowsum reads exp'd AE rows")
            rchf = consts.tile([9, 512], F32, tag="rchunkf", name="rchf", bufs=1)
            nc.vector.reciprocal_approx_fast(rchf[:, 0:N], ps[:, 0:N])
            rch = consts.tile([9, 512], BF16, tag="rchunk", name="rch", bufs=1)
            nc.scalar.copy(rch[:, 0:N], rchf[:, 0:N])
            ps2 = psC.tile([81, 512], F32, tag="rps", name="rps")
            nc.tensor.matmul(ps2[:, 0:N], selrep_bf[:], rch[:, 0:N],
                             start=True, stop=True)
            iv = AE3[:, r0 + 1:r0 + 1 + nr, 1:65]
            nv = nc.vector.tensor_tensor(
                iv, iv, ps2[:, 0:N].rearrange("p (r s) -> p r s", s=66)[:, :, 1:65],
                op=ALU.mult)
            for n8 in range(max(0, r0 // 8), min(8, (r0 + nr) // 8 + 1)):
                add_dep_helper(nv.ins, exp_insts[n8].ins,
                               reason="normalize RMWs exp'd AE rows")
            norm_insts.append(nv)

        # ---- phase D: W stencil build (9 shifted selector matmuls) ----
        cmC.__exit__(None, None, None)
        cmD = tc.tile_pool(name="psD", bufs=8, space="PSUM"); psD = cmD.__enter__()
        W_tap = main.tile([25, L], BF16, tag="wtap", name="W_tap")
        wmask_t = wmask.rearrange("p (u v) -> p v u", u=64)
        wtap_t = W_tap.rearrange("p (u v) -> p v u", u=64)
        ev_insts = []
        first_d = True
        for r0, nr in ROWCH:
            N = nr * 66
            ps = psD.tile([25, 512], F32, tag="wps", name="wps")
            for dd, (di, dj) in enumerate(product(range(3), range(3))):
                st = (r0 + 2 - dj) * 66 + (2 - di)
                mm = nc.tensor.matmul(ps[:, 0:N],
                                      selshift[:, 25 * dd:25 * (dd + 1)],
                                      AE[:, st:st + N],
                                      start=(dd == 0), stop=(dd == 8))
                if first_d:
                    # PE is in-order: gating the first D matmul on all
                    # normalizes covers every later AE read in phase D
                    for nv in norm_insts:
                        add_dep_helper(mm.ins, nv.ins,
                                       reason="D reads normalized AE")
                    first_d = False
            ev_insts.append(nc.vector.tensor_tensor(
                wtap_t[:, r0:r0 + nr, :],
                ps[:, 0:N].rearrange("p (r s) -> p r s", s=66)[:, :, 0:64],
                wmask_t[:, r0:r0 + nr, :], op=ALU.mult))
        cmD.__exit__(None, None, None)

        # ---- phase D2: transpose W to token-major (SBUF only) ----
        cmD2 = tc.tile_pool(name="psD2", bufs=3, space="PSUM"); psD2 = cmD2.__enter__()
        W_tm = main.tile([128, NCHUNK, 25], BF16, tag="wtm", name="W_tm")
        if DEBUG:
            nc.gpsimd.dma_start(d["dbg_wtap"][:], W_tap[:])
        # LDWEIGHTS prefetch hazard: tr's stationary is W_tap, whose writes go
        # through a strided view the tracker misses, and a dep ON tr gates the
        # matmul entry, not its LDW (which executes before the wait).  Gate a
        # PE NOP queue entry on the evacs so no LDW can read W_tap early.
        pe_guard = nc.tensor.nop(nofuse=True, hint="D2 W_tap guard")
        for ev in ev_insts:
            add_dep_helper(pe_guard.ins, ev.ins, reason="D2 reads W_tap")
        wtm_copies = []
        for j in range(NCHUNK):
            pt = psD2.tile([128, 25], BF16, tag="wtp", name="wtp")
            tr = nc.tensor.transpose(pt[:], W_tap[:, 128 * j:128 * (j + 1)],
                                     ident25[:])
            if j == 0:
                add_dep_helper(tr.ins, pe_guard.ins, reason="D2 after guard")
            wtm_copies.append(nc.vector.tensor_copy(W_tm[:, j, :], pt[:]))
        cmD2.__exit__(None, None, None)
        # ---- G^T row-image build, one plain strided store per chunk ----
        # GT[j][m, k] = weight linking source token 128*(j + k//128 - 1) + k%128
        # to output token 128*j + m; tap (e,f) occupies k = m + 64e + f + 128,
        # i.e. flat position 385*m + 64e + f + 128 (+2 global shift so row
        # windows are non-negative).  Row m's 25 taps live at in-row offsets
        # 64*(e+2) + (f+2) of a [128, 384] SBUF image whose gaps are zero, so
        # one strided SBUF copy + one 2D-strided DRAM store per chunk replaces
        # the diagonal scatter.  Geometric clipping (k outside [0, 384)) is
        # exactly the set of taps wmask already zeroed, so no edge cases.
        # The store dst/load src are plain 2D APs the dependency tracker can
        # range-analyze, giving completion-accurate store->load edges.
        gs_t = d["gs"].tensor
        engs = [nc.sync, nc.scalar]
        probe_t = consts.tile([1, 8], F32, tag="probe", name="probe")

        # ---- phase F: maxpools on xT_cm (channel-major grid) ----
        ptmp = es.enter_context(tc.tile_pool(name="ptmp", bufs=3))
        mp_copies = []
        m1 = [main.tile([128, L], BF16, tag=f"m1{cc}", name=f"m1_{cc}") for cc in range(2)]
        m2 = [main.tile([128, L], BF16, tag=f"m2{cc}", name=f"m2_{cc}") for cc in range(2)]

        def g3(ap):
            return ap.rearrange("p (h w) -> p h w", h=64)

        def hmax3(eng, dst, src):
            dv, sv = g3(dst), g3(src)
            t1 = ptmp.tile([128, L], BF16, tag="ptmp", name="ptmp")
            tv = g3(t1)
            eng.tensor_tensor(tv[:, :, 1:], sv[:, :, 1:], sv[:, :, :63], op=ALU.max)
            mp_copies.append(nc.scalar.copy(tv[:, :, 0:1], sv[:, :, 0:1]))
            eng.tensor_tensor(dv[:, :, :63], tv[:, :, :63], sv[:, :, 1:], op=ALU.max)
            mp_copies.append(nc.scalar.copy(dv[:, :, 63:64], tv[:, :, 63:64]))

        def vmax3(eng, dst, src):
            dv, sv = g3(dst), g3(src)
            t1 = ptmp.tile([128, L], BF16, tag="ptmp", name="ptmp")
            tv = g3(t1)
            eng.tensor_tensor(tv[:, 1:, :], sv[:, 1:, :], sv[:, :63, :], op=ALU.max)
            mp_copies.append(nc.scalar.copy(tv[:, 0:1, :], sv[:, 0:1, :]))
            eng.tensor_tensor(dv[:, :63, :], tv[:, :63, :], sv[:, 1:, :], op=ALU.max)
            mp_copies.append(nc.scalar.copy(dv[:, 63:64, :], tv[:, 63:64, :]))

        def hspread(eng, dst, src):   # dst[v] = max(src[v-1], src[v+1]) + edge copies
            dv, sv = g3(dst), g3(src)
            eng.tensor_tensor(dv[:, :, 1:63], sv[:, :, 0:62], sv[:, :, 2:64], op=ALU.max)
            mp_copies.append(nc.scalar.copy(dv[:, :, 0:1], sv[:, :, 1:2]))
            mp_copies.append(nc.scalar.copy(dv[:, :, 63:64], sv[:, :, 62:63]))

        def vspread(eng, dst, src):
            dv, sv = g3(dst), g3(src)
            eng.tensor_tensor(dv[:, 1:63, :], sv[:, 0:62, :], sv[:, 2:64, :], op=ALU.max)
            mp_copies.append(nc.scalar.copy(dv[:, 0:1, :], sv[:, 1:2, :]))
            mp_copies.append(nc.scalar.copy(dv[:, 63:64, :], sv[:, 62:63, :]))

        for cc in range(2):
            eng = nc.vector
            cm3 = ptmp.tile([128, L], BF16, tag="ptmp", name="ptmp")
            hmax3(eng, cm3, xT_cm[cc])
            vmax3(eng, m1[cc], cm3)
            cm5 = ptmp.tile([128, L], BF16, tag="ptmp", name="ptmp")
            hspread(eng, cm5, cm3)
            r35 = ptmp.tile([128, L], BF16, tag="ptmp", name="ptmp")
            vmax3(eng, r35, cm5)
            vspread(eng, m2[cc], r35)

        # ---- phase E: banded stencil apply, c-major out ----
        cmE = tc.tile_pool(name="psE", bufs=2, space="PSUM"); psE = cmE.__enter__()
        x1 = [main.tile([128, L], BF16, tag=f"x1{cc}", name=f"x1_{cc}") for cc in range(2)]
        x2 = [main.tile([128, L], BF16, tag=f"x2{cc}", name=f"x2_{cc}") for cc in range(2)]
        # ONE XBAR load per chunk: G rows (m, b) interleave contiguously at
        # stride 128 (= the XBAR column count), i.e. the transpose input is a
        # plain contiguous [384, 128] block. Column m of source block b then
        # sits at rhs position 3*m + b (stride-3 moving-operand AP).
        last_mm_of_chunk = {}
        gst_of_chunk = {}
        gtr_of_chunk = {}
        for j in range(NCHUNK):
            # build the [128, 384] zero-gapped row image for chunk j and
            # store it as one 2D-strided DMA (row m -> gs[j*GJ + 385*m ..])
            gsb = gpool.tile([128, 384], BF16, tag="gsb", name="gsb", bufs=4)
            if j < 4:
                nc.vector.memset(gsb[:], 0.0)
            cp = nc.vector.tensor_scalar(
                gsb.rearrange("p (g c) -> p g c", c=64)[:, 0:5, 0:5],
                W_tm[:, j, :].rearrange("p (g f) -> p g f", f=5),
                1.0, None, op0=ALU.mult)
            add_dep_helper(cp.ins, wtm_copies[j].ins, reason="gsb reads W_tm")
            if j - 4 in gst_of_chunk:
                add_dep_helper(cp.ins, gst_of_chunk[j - 4].ins,
                               reason="gsb buffer WAR vs store")
            st = nc.scalar.dma_start(
                AP(tensor=gs_t, offset=j * GJ2, ap=[[385, 128], [1, 261]]),
                gsb[:, 0:261])
            add_dep_helper(st.ins, cp.ins, reason="store reads gsb")
            gst_of_chunk[j] = st
            # straight-load the [384, 128] row image as 3 partition blocks and
            # un-transpose on the PE (XBAR transpose-DMA is 5x slower and
            # corrupts under concurrency)
            graw = gpool.tile([128, 3, 128], BF16, tag="graw", name="graw",
                              bufs=4)
            ld = nc.sync.dma_start(
                graw[:],
                AP(tensor=gs_t, offset=j * GJ2 + 2,
                   ap=[[128, 128], [16384, 3], [1, 128]]))
            add_dep_helper(ld.ins, st.ins, reason="G load after store")
            if j - 4 in gtr_of_chunk:
                add_dep_helper(ld.ins, gtr_of_chunk[j - 4].ins,
                               reason="graw buffer WAR")
            g = gpool.tile([128, 384], BF16, tag="g", name="g")
            g3v = g.rearrange("k (m b) -> k m b", b=3)
            gnop = nc.tensor.nop(nofuse=True, hint="G LDW guard")
            add_dep_helper(gnop.ins, ld.ins, reason="G transpose reads load")
            gcps = []
            for b2 in range(3):
                pgt = psE.tile([128, 128], BF16, tag="gtr", name="gtr", bufs=3)
                gtr = nc.tensor.transpose(pgt[:], graw[:, b2, :], ident128[:])
                if b2 == 0:
                    add_dep_helper(gtr.ins, gnop.ins, reason="after LDW guard")
                if b2 < 2:
                    gcp = nc.vector.tensor_copy(
                        g[:, 128 * b2:128 * (b2 + 1)], pgt[:])
                else:
                    gcp = nc.scalar.copy(g[:, 128 * b2:128 * (b2 + 1)], pgt[:])
                if b2 == 0 and j - 8 in last_mm_of_chunk:
                    add_dep_helper(gcp.ins, last_mm_of_chunk[j - 8].ins,
                                   reason="g-buffer WAR")
                gcps.append(gcp)
            gtr_of_chunk[j] = gtr
            bs = [b for b in range(3) if 0 <= j + b - 1 < NCHUNK]
            first_of_chunk = True
            for cc in range(2):
                psx = psE.tile([128, 128], F32, tag=f"psx{cc}", name="psx")
                for i, b in enumerate(bs):
                    mm = nc.tensor.matmul(
                        psx[:],
                        vT[:, j + b - 1, 128 * cc:128 * (cc + 1)],
                        g3v[:, :, b],
                        start=(i == 0), stop=(i == len(bs) - 1))
                    if first_of_chunk:
                        # matmuls read g through the stride-3 view; explicit
                        # edges on the evacs (PE in-order covers the rest;
                        # vector in-order makes gcps[1] cover gcps[0])
                        add_dep_helper(mm.ins, gcps[1].ins,
                                       reason="E matmul reads g evacs")
                        add_dep_helper(mm.ins, gcps[2].ins,
                                       reason="E matmul reads g evacs")
                        first_of_chunk = False
                last_mm_of_chunk[j] = mm
                if cc == 0:
                    nc.scalar.activation(x1[0][:, 128 * j:128 * (j + 1)],
                                         psx[:], ACT.Relu)
                else:
                    nc.vector.tensor_scalar(x1[1][:, 128 * j:128 * (j + 1)],
                                            psx[:], 0.0, None, op0=ALU.max)
        cmE.__exit__(None, None, None)

        if DEBUG:
            for cc in range(2):
                nc.gpsimd.dma_start(d["dbg_x1"][128 * cc:128 * (cc + 1), :], x1[cc][:])
            nc.gpsimd.dma_start(
                d["dbg_vt"][:].rearrange("p (j c) -> p j c", c=C), vT[:])

        # ---- phase G tail: x1 = relu(xr + m1); x2 = relu(x1 + m2) ----
        first_gt = True
        for n8 in range(8):
            sl = slice(512 * n8, 512 * (n8 + 1))
            for cc in range(2):
                gt = nc.vector.tensor_tensor(x1[cc][:, sl], x1[cc][:, sl],
                                             m1[cc][:, sl], op=ALU.add)
                if first_gt:
                    for cp in mp_copies:
                        add_dep_helper(gt.ins, cp.ins,
                                       reason="m-add reads maxpool edge fills")
                    first_gt = False
                nc.scalar.activation(x1[cc][:, sl], x1[cc][:, sl], ACT.Relu)
                nc.vector.tensor_tensor(x2[cc][:, sl], x1[cc][:, sl],
                                        m2[cc][:, sl], op=ALU.add)
                nc.scalar.activation(x2[cc][:, sl], x2[cc][:, sl], ACT.Relu)

        # ---- phase H: fu matmul + residual (mc-outer), BN per half ----
        cmH = tc.tile_pool(name="psH", bufs=4, space="PSUM"); psH = cmH.__enter__()
        out_all = main.tile([128, 2, L], F32, tag="out", name="out_all")
        out_cm = [out_all[:, cc, :] for cc in range(2)]
        small = es.enter_context(tc.tile_pool(name="small", bufs=1))
        bnpack = small.tile([128, 4], F32, tag="bnpack", name="bnpack")
        cin = dram.tile([128, 4], F32, name="cin")
        cout = dram.tile([128, 4], F32, name="cout")
        rhss = [x1[0], x1[1], x2[0], x2[1]]
        for mc in range(2):
            for n8 in range(8):
                sl = slice(512 * n8, 512 * (n8 + 1))
                ps = psH.tile([128, 512], F32, tag="fups", name="fups")
                for kc in range(4):
                    nc.tensor.matmul(ps[:], wfu_sb[:, kc, mc, :],
                                     rhss[kc][:, sl],
                                     start=(kc == 0), stop=(kc == 3))
                nc.scalar.activation(out_cm[mc][:, sl], ps[:], ACT.Relu,
                                     bias=bfu2[:, mc:mc + 1])
                nc.vector.tensor_tensor(out_cm[mc][:, sl], out_cm[mc][:, sl],
                                        xT_cm[mc][:, sl], op=ALU.add)
            st = small.tile([128, 8, 6], F32, tag="bnst", name="bnst")
            for n8 in range(8):
                nc.vector.bn_stats(st[:, n8, :], out_cm[mc][:, 512 * n8:512 * (n8 + 1)])
            ag = small.tile([128, 2], F32, tag="bnag", name="bnag")
            nc.vector.bn_aggr(ag[:], st[:])
            nc.vector.tensor_scalar(bnpack[:, 2 * mc:2 * mc + 1], ag[:, 0:1],
                                    float(L), None, op0=ALU.mult)
            sq = small.tile([128, 1], F32, tag="bnsq", name="bnsq")
            nc.vector.tensor_tensor(sq[:], ag[:, 0:1], ag[:, 0:1], op=ALU.mult)
            nc.vector.tensor_tensor(sq[:], sq[:], ag[:, 1:2], op=ALU.add)
            nc.vector.tensor_scalar(bnpack[:, 2 * mc + 1:2 * mc + 2], sq[:],
                                    float(L), None, op0=ALU.mult)
        # one packed AllReduce for both halves' (sum, sumsq)
        nc.sync.dma_start(cin[:], bnpack[:])
        nc.gpsimd.collective_compute(
            "AllReduce", ALU.add,
            replica_groups=[list(range(n_cores))],
            ins=[cin.opt()], outs=[cout.opt()])
        gs_sb = small.tile([128, 4], F32, tag="gsb", name="gs_sb")
        nc.sync.dma_start(gs_sb[:], cout[:])
        NTOT = float(n_cores * L)
        scale = small.tile([128, 2], F32, tag="scale", name="scale")
        shift = small.tile([128, 2], F32, tag="shift", name="shift")
        mean = small.tile([128, 2], F32, tag="mean", name="mean")
        var = small.tile([128, 2], F32, tag="var", name="var")
        for cc in range(2):
            nc.vector.tensor_scalar(mean[:, cc:cc + 1], gs_sb[:, 2 * cc:2 * cc + 1],
                                    1.0 / NTOT, None, op0=ALU.mult)
            nc.vector.tensor_scalar(var[:, cc:cc + 1], gs_sb[:, 2 * cc + 1:2 * cc + 2],
                                    1.0 / NTOT, None, op0=ALU.mult)
        msq = small.tile([128, 2], F32, tag="msq", name="msq")
        nc.vector.tensor_tensor(msq[:], mean[:], mean[:], op=ALU.mult)
        nc.vector.tensor_tensor(var[:], var[:], msq[:], op=ALU.subtract)
        rs = small.tile([128, 2], F32, tag="rs", name="rs")
        nc.vector.tensor_scalar(var[:], var[:], float(EPS), None, op0=ALU.add)
        nc.scalar.activation(rs[:], var[:], ACT.Sqrt)
        nc.vector.reciprocal(rs[:], rs[:])
        nc.vector.tensor_tensor(scale[:], gamma2[:], rs[:], op=ALU.mult)
        nc.vector.tensor_tensor(shift[:], mean[:], scale[:], op=ALU.mult)
        nc.vector.tensor_tensor(shift[:], beta2[:], shift[:], op=ALU.subtract)

        if DEBUG2:
            nc.gpsimd.dma_start(
                d["dbg_vt"][:].rearrange("p (j c) -> p j c", c=C), vT[:])
            nc.gpsimd.dma_start(d["dbg_wtap"][:], W_tap[:])
            nc.gpsimd.dma_start(
                d["dbg_wtm"][:].rearrange("p (j t) -> p j t", t=25), W_tm[:])
            nc.gpsimd.dma_start(d["dbg_ae"][:], AE[:])
            nc.gpsimd.dma_start(d["dbg_gs"][:], d["gs"][0:NCHUNK * GJ2])
            nc.gpsimd.dma_start(
                d["dbg_out"][:].rearrange("p (m l) -> p m l", l=L), out_all[:])
            for cc in range(2):
                nc.gpsimd.dma_start(d["dbg_xcm"][:, L * cc:L * (cc + 1)],
                                    xT_cm[cc][:])

        # normalize into the dead x1 tiles (bf16), DMA out c-major
        # (host un-transposes and upcasts)
        for n8 in range(8):
            sl = slice(512 * n8, 512 * (n8 + 1))
            for cc in range(2):
                if cc == 0:
                    nc.vector.tensor_scalar(x1[0][:, sl], out_cm[0][:, sl],
                                            scale[:, 0:1], shift[:, 0:1],
                                            op0=ALU.mult, op1=ALU.add)
                else:
                    nc.scalar.activation(x1[1][:, sl], out_cm[1][:, sl],
                                         ACT.Identity, bias=shift[:, 1:2],
                                         scale=scale[:, 1:2])
                eng = nc.sync if (n8 % 2 == 0) else nc.scalar
                eng.dma_start(d["y"][128 * cc:128 * (cc + 1), sl],
                              x1[cc][:, sl])
        cmH.__exit__(None, None, None)


_CACHE = {}


def _get_program(n_cores=N_CORES):
    key = n_cores
    if key not in _CACHE:
        nc = bacc.Bacc("TRN2", target_bir_lowering=False, debug=False,
                       num_devices=n_cores)
        build(nc, n_cores)
        nc.compile()
        _CACHE[key] = nc
    return _CACHE[key]


_CONSTS = None


def make_in_map(inputs, b):
    global _CONSTS
    if _CONSTS is None:
        _CONSTS = host_consts()
    import ml_dtypes
    # pre-permute to the transposed-grid token order l' = w*64 + h
    xbf = np.ascontiguousarray(
        np.asarray(inputs["x"][b]).transpose(1, 0, 2).reshape(L, C)
    ).astype(ml_dtypes.bfloat16)
    return {
        "xbf": xbf,
        "wv": np.ascontiguousarray(inputs["Wv"], np.float32),
        "bv": np.ascontiguousarray(np.asarray(inputs["bv"]).reshape(1, C), np.float32),
        "wa": np.ascontiguousarray(inputs["Wa"], np.float32),
        "ba": np.ascontiguousarray(np.asarray(inputs["ba"]).reshape(81, 1), np.float32),
        "wfu": np.ascontiguousarray(inputs["Wfu"], np.float32),
        "bfu2": np.ascontiguousarray(
            np.asarray(inputs["bfu"]).reshape(2, 128).T, np.float32),
        "gamma2": np.ascontiguousarray(
            np.asarray(inputs["gamma"]).reshape(2, 128).T, np.float32),
        "beta2": np.ascontiguousarray(
            np.asarray(inputs["beta"]).reshape(2, 128).T, np.float32),
        **_CONSTS,
    }


def postprocess(yarr):
    """[256, L] c-major, l' = w*64+h  ->  [H, W, C] in the reference frame."""
    return np.asarray(yarr, np.float32).reshape(C, L).T.reshape(H, W, C)


def kernel(**inputs):
    nc = _get_program()
    in_maps = [make_in_map(inputs, b) for b in range(B)]
    res = run_bass_kernel_spmd(nc, in_maps, list(range(N_CORES)))
    out = np.stack([postprocess(res.results[b]["y"]) for b in range(B)])
    return out.astype(np.float32)



# revision 63
# speedup vs baseline: 1.5191x; 1.0044x over previous
"""Trainium2 Bass kernel for nn_MOA_13254269075617 (sparse windowed attention block).

Sharding: data-parallel over batch B=8 across 8 NeuronCores (1 image each).
BatchNorm uses global batch stats via an on-device AllReduce of per-channel
sum / sum-of-squares.

Per-core pipeline (all in the spatially-TRANSPOSED frame; host pre-permutes
the input to token order l' = w*64 + h and un-permutes the c-major output):
  xT_cm  : x channel-major [2x128, 4096] via transpose-DMAs
  vT     : (x @ Wv + bv) token-major [128, 32, 256] bf16
  A      : softmax(x @ Wa + ba) pq-major [81, 4096] on a zero-padded grid
  W_tap  : 25-tap position-varying stencil weights [25, 4096] (fold+attention
           combined algebraically), wmask'd at grid edges
  G      : banded token->token weight matrices assembled in DRAM by strided
           scatter-DMA (5-tap f-runs = 10B descriptors), stored transposed
           [m, k] and un-transposed by the DMA XBAR on load
  xf     : stencil apply = 6 PE matmuls per 128-token chunk,
           out[c, m] = sum_k vT[k, c] * G[k, m], PSUM-accumulated c-major
  x1/x2  : relu chains with 3x3/5x5 maxpools (separable shifted-max trees)
  out    : concat-matmul (Wfu) + residual, BN with AllReduce'd stats,
           written c-major [256, 4096]; host transposes back
"""
import sys

for _p in (
    "/root/.axon_site",
    "/root/.axon_site/_ro/trn_rl_repo",
    "/root/.axon_site/_ro/pypackages",
    "/opt/trn_rl_repo",
):
    if _p not in sys.path:
        sys.path.append(_p)

from itertools import product

import numpy as np

import concourse.bass as bass
import concourse.tile as tile
from concourse.ap import AP
from concourse import bacc, mybir
from concourse.bass_utils import run_bass_kernel_spmd

F32 = mybir.dt.float32
BF16 = mybir.dt.bfloat16
ALU = mybir.AluOpType
ACT = mybir.ActivationFunctionType

import os
DEBUG2 = os.environ.get("KDEBUG") == "1"
B, H, W, C = 8, 64, 64, 256
L = H * W                      # 4096 tokens
NCHUNK = L // 128              # 32 token chunks
N_CORES = 8
EPS = 1e-5
GROW = 384                     # 3 source blocks x 128 rows per chunk
GJ = GROW * 128                # G elems per chunk
GJ2 = GJ + 128                 # chunk stride in gs: +128 so row images of
                               # adjacent chunks never overlap (no WAR chains)
GSPAD = NCHUNK * GJ2 + GJ + 4096  # guard for conservative OOB checks
DEBUG = False
TAPS = [(e, f) for e in range(-2, 3) for f in range(-2, 3)]


def host_consts():
    """Selector matrices and small constants (host-precomputed, same all cores)."""
    selsum = np.zeros((81, 9), np.float32)
    for p in range(9):
        selsum[9 * p:9 * p + 9, p] = 1.0
    selrep = np.zeros((9, 81), np.float32)
    for p in range(9):
        selrep[p, 9 * p:9 * p + 9] = 1.0
    # selshift[:, 25*d + tap]: for (di,dj) block d, tap (e,f):
    #   k = 9*(3di+dj) + 3(di+e)+(dj+f) if di+e,dj+f in [0,3)
    selshift = np.zeros((81, 9 * 25), np.float32)
    for d, (di, dj) in enumerate(product(range(3), range(3))):
        for t, (e, f) in enumerate(product(range(-2, 3), range(-2, 3))):
            dip, djp = di + e, dj + f
            if 0 <= dip < 3 and 0 <= djp < 3:
                k = 9 * (3 * di + dj) + (3 * dip + djp)
                selshift[k, 25 * d + t] = 1.0
    wmask = np.ones((25, 64, 64), np.float32)
    for t, (e, f) in enumerate(product(range(-2, 3), range(-2, 3))):
        if e > 0: wmask[t, 64 - e:, :] = 0
        if e < 0: wmask[t, :-e, :] = 0
        if f > 0: wmask[t, :, 64 - f:] = 0
        if f < 0: wmask[t, :, :-f] = 0
    import ml_dtypes
    return {
        "selsum": selsum,
        "selrep": selrep,
        "selshift": selshift,
        "wmask": wmask.reshape(25, 4096),
        "ident25": np.eye(25, dtype=np.float32),
        "ident128": np.eye(128, dtype=np.float32),
        "ones1": np.ones((1, 128), np.float32),
        "gs": np.zeros(GSPAD, dtype=ml_dtypes.bfloat16),
    }


def build(nc, n_cores):
    d = {}
    def din(name, shape):
        d[name] = nc.dram_tensor(name, list(shape), F32, kind="ExternalInput").ap()

    d["xbf"] = nc.dram_tensor("xbf", [L, C], BF16, kind="ExternalInput").ap()
    d["gs"] = nc.dram_tensor("gs", [GSPAD], BF16, kind="ExternalInput").ap()
    din("wv", (C, C)); din("bv", (1, C))
    din("wa", (C, 81)); din("ba", (81, 1))
    din("wfu", (2 * C, C)); din("bfu2", (128, 2))
    din("gamma2", (128, 2)); din("beta2", (128, 2))
    din("selsum", (81, 9)); din("selrep", (9, 81)); din("selshift", (81, 225))
    din("ident25", (25, 25)); din("ident128", (128, 128))
    din("ones1", (1, 128)); din("wmask", (25, L))
    d["y"] = nc.dram_tensor("y", [2 * 128, L], BF16, kind="ExternalOutput").ap()
    if DEBUG2:
        d["dbg_vt"] = nc.dram_tensor("dbg_vt", [128, NCHUNK * C], BF16, kind="ExternalOutput").ap()
        d["dbg_wtap"] = nc.dram_tensor("dbg_wtap", [25, L], BF16, kind="ExternalOutput").ap()
        d["dbg_wtm"] = nc.dram_tensor("dbg_wtm", [128, NCHUNK * 25], BF16, kind="ExternalOutput").ap()
        d["dbg_ae"] = nc.dram_tensor("dbg_ae", [81, 66 * 67], BF16, kind="ExternalOutput").ap()
        d["dbg_gs"] = nc.dram_tensor("dbg_gs", [NCHUNK * GJ2], BF16, kind="ExternalOutput").ap()
        d["dbg_out"] = nc.dram_tensor("dbg_out", [128, 2 * L], BF16, kind="ExternalOutput").ap()
        d["dbg_xcm"] = nc.dram_tensor("dbg_xcm", [128, 2 * L], BF16, kind="ExternalOutput").ap()
    if DEBUG:
        d["dbg_wtap"] = nc.dram_tensor("dbg_wtap", [25, L], F32, kind="ExternalOutput").ap()
        d["dbg_wtm"] = nc.dram_tensor("dbg_wtm", [128, NCHUNK * 25], F32, kind="ExternalOutput").ap()
        d["dbg_wtmd"] = nc.dram_tensor("dbg_wtmd", [L * 25], BF16, kind="ExternalOutput").ap()
        d["dbg_gs"] = nc.dram_tensor("dbg_gs", [4 * GJ], BF16, kind="ExternalOutput").ap()
        d["dbg_x1"] = nc.dram_tensor("dbg_x1", [2 * 128, L], F32, kind="ExternalOutput").ap()
        d["dbg_vt"] = nc.dram_tensor("dbg_vt", [128, NCHUNK * C], F32, kind="ExternalOutput").ap()

    with tile.TileContext(nc) as tc:
        _build_tc(tc, d, n_cores)
    return d


def _build_tc(tc, d, n_cores):
    nc = tc.nc
    from contextlib import ExitStack
    es = ExitStack()
    with es:
        consts = es.enter_context(tc.tile_pool(name="consts", bufs=1))
        main = es.enter_context(tc.tile_pool(name="main", bufs=1))
        gpool = es.enter_context(tc.tile_pool(name="gpool", bufs=8))
        dram = es.enter_context(tc.tile_pool(name="dram", bufs=2, space="DRAM"))

        # ---- const loads ----
        def cload(name, shape):
            t = consts.tile(list(shape), F32, tag=name, name=name)
            nc.gpsimd.dma_start(t[:], d[name][:])
            return t
        def cload_bf(name, shape):
            t = consts.tile(list(shape), BF16, tag=name, name=name)
            nc.gpsimd.dma_start(t[:], d[name][:])
            return t
        # order by first use: phase A needs ident128, B needs ones1/bv/wv,
        # C needs wa/ba, then selectors, then late-phase consts
        ident128 = cload_bf("ident128", (128, 128))
        ones1 = cload_bf("ones1", (1, 128))
        bv_sb = cload_bf("bv", (1, C))
        wv_sb = consts.tile([128, 2, C], BF16, tag="wv", name="wv_sb")
        for kc in range(2):
            nc.gpsimd.dma_start(wv_sb[:, kc, :], d["wv"][128 * kc:128 * (kc + 1), :])
        wa_sb = consts.tile([128, 2, 81], BF16, tag="wa", name="wa_sb")
        for kc in range(2):
            nc.gpsimd.dma_start(wa_sb[:, kc, :], d["wa"][128 * kc:128 * (kc + 1), :])
        ba_sb = cload("ba", (81, 1))
        selsum = cload_bf("selsum", (81, 9))
        selrep_bf = cload_bf("selrep", (9, 81))
        selshift = cload_bf("selshift", (81, 225))
        ident25 = cload_bf("ident25", (25, 25))
        wmask = main.tile([25, L], BF16, tag="wmask", name="wmask")
        nc.gpsimd.dma_start(wmask[:], d["wmask"][:])
        wfu_sb = consts.tile([128, 4, 2, 128], BF16, tag="wfu", name="wfu_sb")
        for kc in range(4):
            for mc in range(2):
                nc.gpsimd.dma_start(
                    wfu_sb[:, kc, mc, :],
                    d["wfu"][128 * kc:128 * (kc + 1), 128 * mc:128 * (mc + 1)])
        bfu2 = cload("bfu2", (128, 2))
        gamma2 = cload("gamma2", (128, 2))
        beta2 = cload("beta2", (128, 2))

        # ---- phase A: straight-load x token-major, PE-transpose to c-major ----
        # (host pre-permuted xbf rows to l' = w*64 + h).  XBAR transpose-DMAs
        # run at ~27GB/s on a single SDMA engine and corrupt each other when
        # two run concurrently; straight DMA + PE transpose is ~5x faster and
        # uses otherwise-idle PE time.
        from concourse.tile import add_dep_helper
        engs = [nc.sync, nc.scalar]
        xT_cm = [main.tile([128, L], BF16, tag=f"xcm{cc}", name=f"xT_cm{cc}")
                 for cc in range(2)]
        cmA = tc.tile_pool(name="psA", bufs=4, space="PSUM"); psA = cmA.__enter__()
        a_trs = {}
        for j in range(NCHUNK):
            xtm = gpool.tile([128, C], BF16, tag="xtm", name="xtm", bufs=4)
            dmx = engs[j % 2].dma_start(xtm[:], d["xbf"][128 * j:128 * (j + 1), :])
            if j - 4 in a_trs:
                add_dep_helper(dmx.ins, a_trs[j - 4].ins, reason="xtm WAR")
            for cc in range(2):
                pa = psA.tile([128, 128], BF16, tag="psa", name="psa")
                tr = nc.tensor.transpose(
                    pa[:], xtm[:, 128 * cc:128 * (cc + 1)], ident128[:])
                if cc == 0:
                    nc.scalar.copy(xT_cm[0][:, 128 * j:128 * (j + 1)], pa[:])
                else:
                    nc.vector.tensor_copy(xT_cm[1][:, 128 * j:128 * (j + 1)],
                                          pa[:])
            a_trs[j] = tr
        cmA.__exit__(None, None, None)

        # ---- phase B: vT = xT @ Wv + bv, token-major (bf16) ----
        cmAB = tc.tile_pool(name="psAB", bufs=3, space="PSUM"); psAB = cmAB.__enter__()
        vT = main.tile([128, NCHUNK, C], BF16, tag="vT", name="vT")
        bv256 = consts.tile([128, C], BF16, tag="bv256", name="bv256")
        psb = psAB.tile([128, C], F32, tag="vps", name="vps")
        nc.tensor.matmul(psb[:], ones1[:], bv_sb[:], start=True, stop=True)
        nc.scalar.copy(bv256[:], psb[:])
        for j in range(NCHUNK):
            ps = psAB.tile([128, C], F32, tag="vps", name="vps")
            nc.tensor.matmul(ps[:], xT_cm[0][:, 128 * j:128 * (j + 1)],
                             wv_sb[:, 0, :], start=True, stop=False)
            nc.tensor.matmul(ps[:], xT_cm[1][:, 128 * j:128 * (j + 1)],
                             wv_sb[:, 1, :], start=False, stop=True)
            nc.vector.tensor_tensor(vT[:, j, :], ps[:], bv256[:], op=ALU.add)

        # ---- phase C: attention logits -> exp -> normalize ----
        cmAB.__exit__(None, None, None)
        cmC = tc.tile_pool(name="psC", bufs=2, space="PSUM"); psC = cmC.__enter__()
        AE = main.tile([81, 66 * 67], BF16, tag="AE", name="AE")
        nc.gpsimd.memset(AE[:], 0.0)
        AE3 = AE.rearrange("p (r s) -> p r s", r=67)

        # NOTE: the automatic dependency tracker is unreliable for the strided
        # AE3 views, so cross-engine RAW edges here are added explicitly
        # (per-engine in-order execution covers the downstream instructions).
        from concourse.tile import add_dep_helper
        exp_insts = []
        for n8 in range(8):
            ps = psC.tile([81, 512], F32, tag="aps", name="aps")
            for kc in range(2):
                nc.tensor.matmul(ps[:], wa_sb[:, kc, :],
                                 xT_cm[kc][:, 512 * n8:512 * (n8 + 1)],
                                 start=(kc == 0), stop=(kc == 1))
            exp_insts.append(nc.scalar.activation(
                AE3[:, 1 + 8 * n8:1 + 8 * n8 + 8, 1:65],
                ps.rearrange("p (r s) -> p r s", s=64),
                ACT.Exp, bias=ba_sb[:, 0:1]))
        ROWCH = [(r0, min(7, 64 - r0)) for r0 in range(0, 64, 7)]
        norm_insts = []
        for ci, (r0, nr) in enumerate(ROWCH):
            N = nr * 66
            win = slice((r0 + 1) * 66, (r0 + 1) * 66 + N)
            ps = psC.tile([9, 512], F32, tag="sps", name="sps")
            mm = nc.tensor.matmul(ps[:, 0:N], selsum[:], AE[:, win],
                                  start=True, stop=True)
            # rowsum reads AE rows [r0+1, r0+1+nr): wait for the exp blocks
            for n8 in range(max(0, r0 // 8), min(8, (r0 + nr) // 8 + 1)):
                add_dep_helper(mm.ins, exp_insts[n8].ins,
                               reason="rowsum reads exp'd AE rows")
            rchf = consts.tile([9, 512], F32, tag="rchunkf", name="rchf", bufs=1)
            nc.vector.reciprocal_approx_fast(rchf[:, 0:N], ps[:, 0:N])
            rch = consts.tile([9, 512], BF16, tag="rchunk", name="rch", bufs=1)
            nc.scalar.copy(rch[:, 0:N], rchf[:, 0:N])
            ps2 = psC.tile([81, 512], F32, tag="rps", name="rps")
            nc.tensor.matmul(ps2[:, 0:N], selrep_bf[:], rch[:, 0:N],
                             start=True, stop=True)
            iv = AE3[:, r0 + 1:r0 + 1 + nr, 1:65]
            nv = nc.vector.tensor_tensor(
                iv, iv, ps2[:, 0:N].rearrange("p (r s) -> p r s", s=66)[:, :, 1:65],
                op=ALU.mult)
            for n8 in range(max(0, r0 // 8), min(8, (r0 + nr) // 8 + 1)):
                add_dep_helper(nv.ins, exp_insts[n8].ins,
                               reason="normalize RMWs exp'd AE rows")
            norm_insts.append(nv)

        # ---- phase D: W stencil build (9 shifted selector matmuls) ----
        cmC.__exit__(None, None, None)
        cmD = tc.tile_pool(name="psD", bufs=8, space="PSUM"); psD = cmD.__enter__()
        W_tap = main.tile([25, L], BF16, tag="wtap", name="W_tap")
        wmask_t = wmask.rearrange("p (u v) -> p v u", u=64)
        wtap_t = W_tap.rearrange("p (u v) -> p v u", u=64)
        ev_insts = []
        for r0, nr in ROWCH:
            N = nr * 66
            ps = psD.tile([25, 512], F32, tag="wps", name="wps")
            for dd, (di, dj) in enumerate(product(range(3), range(3))):
                st = (r0 + 2 - dj) * 66 + (2 - di)
                mm = nc.tensor.matmul(ps[:, 0:N],
                                      selshift[:, 25 * dd:25 * (dd + 1)],
                                      AE[:, st:st + N],
                                      start=(dd == 0), stop=(dd == 8))
                if dd == 0:
                    # this rowchunk reads AE grid rows [r0, r0+nr+1]; gate on
                    # just the normalizes covering those rows so D interleaves
                    # with the (serial) softmax-normalize chain
                    for (rn, nrn), nv in zip(ROWCH, norm_insts):
                        if rn + 1 <= r0 + nr + 1 and rn + nrn >= r0:
                            add_dep_helper(mm.ins, nv.ins,
                                           reason="D reads normalized AE")
            ev_insts.append(nc.vector.tensor_tensor(
                wtap_t[:, r0:r0 + nr, :],
                ps[:, 0:N].rearrange("p (r s) -> p r s", s=66)[:, :, 0:64],
                wmask_t[:, r0:r0 + nr, :], op=ALU.mult))
        cmD.__exit__(None, None, None)

        # ---- phase D2: transpose W to token-major (SBUF only) ----
        cmD2 = tc.tile_pool(name="psD2", bufs=3, space="PSUM"); psD2 = cmD2.__enter__()
        W_tm = main.tile([128, NCHUNK, 25], BF16, tag="wtm", name="W_tm")
        if DEBUG:
            nc.gpsimd.dma_start(d["dbg_wtap"][:], W_tap[:])
        # LDWEIGHTS prefetch hazard: tr's stationary is W_tap, whose writes go
        # through a strided view the tracker misses, and a dep ON tr gates the
        # matmul entry, not its LDW (which executes before the wait).  Gate a
        # PE NOP queue entry on the evacs so no LDW can read W_tap early.
        pe_guard = nc.tensor.nop(nofuse=True, hint="D2 W_tap guard")
        for ev in ev_insts:
            add_dep_helper(pe_guard.ins, ev.ins, reason="D2 reads W_tap")
        wtm_copies = []
        for j in range(NCHUNK):
            pt = psD2.tile([128, 25], BF16, tag="wtp", name="wtp")
            tr = nc.tensor.transpose(pt[:], W_tap[:, 128 * j:128 * (j + 1)],
                                     ident25[:])
            if j == 0:
                add_dep_helper(tr.ins, pe_guard.ins, reason="D2 after guard")
            wtm_copies.append(nc.vector.tensor_copy(W_tm[:, j, :], pt[:]))
        cmD2.__exit__(None, None, None)
        # ---- G^T row-image build, one plain strided store per chunk ----
        # GT[j][m, k] = weight linking source token 128*(j + k//128 - 1) + k%128
        # to output token 128*j + m; tap (e,f) occupies k = m + 64e + f + 128,
        # i.e. flat position 385*m + 64e + f + 128 (+2 global shift so row
        # windows are non-negative).  Row m's 25 taps live at in-row offsets
        # 64*(e+2) + (f+2) of a [128, 384] SBUF image whose gaps are zero, so
        # one strided SBUF copy + one 2D-strided DRAM store per chunk replaces
        # the diagonal scatter.  Geometric clipping (k outside [0, 384)) is
        # exactly the set of taps wmask already zeroed, so no edge cases.
        # The store dst/load src are plain 2D APs the dependency tracker can
        # range-analyze, giving completion-accurate store->load edges.
        gs_t = d["gs"].tensor
        engs = [nc.sync, nc.scalar]
        probe_t = consts.tile([1, 8], F32, tag="probe", name="probe")

        # ---- phase F: maxpools on xT_cm (channel-major grid) ----
        ptmp = es.enter_context(tc.tile_pool(name="ptmp", bufs=3))
        mp_copies = []
        m1 = [main.tile([128, L], BF16, tag=f"m1{cc}", name=f"m1_{cc}") for cc in range(2)]
        m2 = [main.tile([128, L], BF16, tag=f"m2{cc}", name=f"m2_{cc}") for cc in range(2)]

        def g3(ap):
            return ap.rearrange("p (h w) -> p h w", h=64)

        def hmax3(eng, dst, src, tg):
            dv, sv = g3(dst), g3(src)
            t1 = ptmp.tile([128, L], BF16, tag=tg, name="ptmp", bufs=3)
            tv = g3(t1)
            eng.tensor_tensor(tv[:, :, 1:], sv[:, :, 1:], sv[:, :, :63], op=ALU.max)
            mp_copies.append(nc.scalar.copy(tv[:, :, 0:1], sv[:, :, 0:1]))
            eng.tensor_tensor(dv[:, :, :63], tv[:, :, :63], sv[:, :, 1:], op=ALU.max)
            mp_copies.append(nc.scalar.copy(dv[:, :, 63:64], tv[:, :, 63:64]))

        def vmax3(eng, dst, src, tg):
            dv, sv = g3(dst), g3(src)
            t1 = ptmp.tile([128, L], BF16, tag=tg, name="ptmp", bufs=3)
            tv = g3(t1)
            eng.tensor_tensor(tv[:, 1:, :], sv[:, 1:, :], sv[:, :63, :], op=ALU.max)
            mp_copies.append(nc.scalar.copy(tv[:, 0:1, :], sv[:, 0:1, :]))
            eng.tensor_tensor(dv[:, :63, :], tv[:, :63, :], sv[:, 1:, :], op=ALU.max)
            mp_copies.append(nc.scalar.copy(dv[:, 63:64, :], tv[:, 63:64, :]))

        def hspread(eng, dst, src):   # dst[v] = max(src[v-1], src[v+1]) + edge copies
            dv, sv = g3(dst), g3(src)
            eng.tensor_tensor(dv[:, :, 1:63], sv[:, :, 0:62], sv[:, :, 2:64], op=ALU.max)
            mp_copies.append(nc.scalar.copy(dv[:, :, 0:1], sv[:, :, 1:2]))
            mp_copies.append(nc.scalar.copy(dv[:, :, 63:64], sv[:, :, 62:63]))

        def vspread(eng, dst, src):
            dv, sv = g3(dst), g3(src)
            eng.tensor_tensor(dv[:, 1:63, :], sv[:, 0:62, :], sv[:, 2:64, :], op=ALU.max)
            mp_copies.append(nc.scalar.copy(dv[:, 0:1, :], sv[:, 1:2, :]))
            mp_copies.append(nc.scalar.copy(dv[:, 63:64, :], sv[:, 62:63, :]))

        for cc in range(2):
            eng = nc.vector
            tg = f"ptmp{cc}"
            cm3 = ptmp.tile([128, L], BF16, tag=tg, name="ptmp", bufs=3)
            hmax3(eng, cm3, xT_cm[cc], tg)
            vmax3(eng, m1[cc], cm3, tg)
            cm5 = ptmp.tile([128, L], BF16, tag=tg, name="ptmp", bufs=3)
            hspread(eng, cm5, cm3)
            r35 = ptmp.tile([128, L], BF16, tag=tg, name="ptmp", bufs=3)
            vmax3(eng, r35, cm5, tg)
            vspread(eng, m2[cc], r35)

        # ---- phase E: banded stencil apply, c-major out ----
        cmE = tc.tile_pool(name="psE", bufs=2, space="PSUM"); psE = cmE.__enter__()
        x1 = [main.tile([128, L], BF16, tag=f"x1{cc}", name=f"x1_{cc}") for cc in range(2)]
        x2 = [main.tile([128, L], BF16, tag=f"x2{cc}", name=f"x2_{cc}") for cc in range(2)]
        # ONE XBAR load per chunk: G rows (m, b) interleave contiguously at
        # stride 128 (= the XBAR column count), i.e. the transpose input is a
        # plain contiguous [384, 128] block. Column m of source block b then
        # sits at rhs position 3*m + b (stride-3 moving-operand AP).
        last_mm_of_chunk = {}
        gst_of_chunk = {}
        gtr_of_chunk = {}
        for j in range(NCHUNK):
            # build the [128, 384] zero-gapped row image for chunk j and
            # store it as one 2D-strided DMA (row m -> gs[j*GJ + 385*m ..])
            gsb = gpool.tile([128, 384], BF16, tag="gsb", name="gsb", bufs=3)
            if j < 4:
                nc.vector.memset(gsb[:], 0.0)
            cp = nc.vector.tensor_scalar(
                gsb.rearrange("p (g c) -> p g c", c=64)[:, 0:5, 0:5],
                W_tm[:, j, :].rearrange("p (g f) -> p g f", f=5),
                1.0, None, op0=ALU.mult)
            add_dep_helper(cp.ins, wtm_copies[j].ins, reason="gsb reads W_tm")
            if j - 3 in gst_of_chunk:
                add_dep_helper(cp.ins, gst_of_chunk[j - 3].ins,
                               reason="gsb buffer WAR vs store")
            st = nc.scalar.dma_start(
                AP(tensor=gs_t, offset=j * GJ2, ap=[[385, 128], [1, 261]]),
                gsb[:, 0:261])
            add_dep_helper(st.ins, cp.ins, reason="store reads gsb")
            gst_of_chunk[j] = st
            # straight-load the [384, 128] row image as 3 partition blocks and
            # un-transpose on the PE (XBAR transpose-DMA is 5x slower and
            # corrupts under concurrency)
            graw = gpool.tile([128, 3, 128], BF16, tag="graw", name="graw",
                              bufs=4)
            ld = nc.sync.dma_start(
                graw[:],
                AP(tensor=gs_t, offset=j * GJ2 + 2,
                   ap=[[128, 128], [16384, 3], [1, 128]]))
            add_dep_helper(ld.ins, st.ins, reason="G load after store")
            if j - 4 in gtr_of_chunk:
                add_dep_helper(ld.ins, gtr_of_chunk[j - 4].ins,
                               reason="graw buffer WAR")
            g = gpool.tile([128, 384], BF16, tag="g", name="g", bufs=6)
            g3v = g.rearrange("k (m b) -> k m b", b=3)
            gnop = nc.tensor.nop(nofuse=True, hint="G LDW guard")
            add_dep_helper(gnop.ins, ld.ins, reason="G transpose reads load")
            gcps = []
            for b2 in range(3):
                pgt = psE.tile([128, 128], BF16, tag="gtr", name="gtr", bufs=3)
                gtr = nc.tensor.transpose(pgt[:], graw[:, b2, :], ident128[:])
                if b2 == 0:
                    add_dep_helper(gtr.ins, gnop.ins, reason="after LDW guard")
                if b2 < 2:
                    gcp = nc.vector.tensor_copy(
                        g[:, 128 * b2:128 * (b2 + 1)], pgt[:])
                else:
                    gcp = nc.scalar.copy(g[:, 128 * b2:128 * (b2 + 1)], pgt[:])
                if b2 == 0 and j - 6 in last_mm_of_chunk:
                    add_dep_helper(gcp.ins, last_mm_of_chunk[j - 6].ins,
                                   reason="g-buffer WAR")
                gcps.append(gcp)
            gtr_of_chunk[j] = gtr
            bs = [b for b in range(3) if 0 <= j + b - 1 < NCHUNK]
            first_of_chunk = True
            for cc in range(2):
                psx = psE.tile([128, 128], F32, tag=f"psx{cc}", name="psx")
                for i, b in enumerate(bs):
                    mm = nc.tensor.matmul(
                        psx[:],
                        vT[:, j + b - 1, 128 * cc:128 * (cc + 1)],
                        g3v[:, :, b],
                        start=(i == 0), stop=(i == len(bs) - 1))
                    if first_of_chunk:
                        # matmuls read g through the stride-3 view; explicit
                        # edges on the evacs (PE in-order covers the rest;
                        # vector in-order makes gcps[1] cover gcps[0])
                        add_dep_helper(mm.ins, gcps[1].ins,
                                       reason="E matmul reads g evacs")
                        add_dep_helper(mm.ins, gcps[2].ins,
                                       reason="E matmul reads g evacs")
                        first_of_chunk = False
                last_mm_of_chunk[j] = mm
                if cc == 0:
                    nc.scalar.activation(x1[0][:, 128 * j:128 * (j + 1)],
                                         psx[:], ACT.Relu)
                else:
                    nc.vector.tensor_scalar(x1[1][:, 128 * j:128 * (j + 1)],
                                            psx[:], 0.0, None, op0=ALU.max)
        cmE.__exit__(None, None, None)

        if DEBUG:
            for cc in range(2):
                nc.gpsimd.dma_start(d["dbg_x1"][128 * cc:128 * (cc + 1), :], x1[cc][:])
            nc.gpsimd.dma_start(
                d["dbg_vt"][:].rearrange("p (j c) -> p j c", c=C), vT[:])

        # ---- phase G tail: x1 = relu(xr + m1); x2 = relu(x1 + m2) ----
        first_gt = {0: True, 1: True}
        for n8 in range(8):
            sl = slice(512 * n8, 512 * (n8 + 1))
            for cc in range(2):
                ee = nc.vector
                gt = ee.tensor_tensor(x1[cc][:, sl], x1[cc][:, sl],
                                      m1[cc][:, sl], op=ALU.add)
                if first_gt[cc]:
                    for cp in mp_copies:
                        add_dep_helper(gt.ins, cp.ins,
                                       reason="m-add reads maxpool edge fills")
                    first_gt[cc] = False
                nc.scalar.activation(x1[cc][:, sl], x1[cc][:, sl], ACT.Relu)
                ee.tensor_tensor(x2[cc][:, sl], x1[cc][:, sl],
                                 m2[cc][:, sl], op=ALU.add)
                nc.scalar.activation(x2[cc][:, sl], x2[cc][:, sl], ACT.Relu)

        # ---- phase H: fu matmul + residual (mc-outer), BN per half ----
        cmH = tc.tile_pool(name="psH", bufs=4, space="PSUM"); psH = cmH.__enter__()
        out_all = main.tile([128, 2, L], BF16, tag="out", name="out_all")
        out_cm = [out_all[:, cc, :] for cc in range(2)]
        small = es.enter_context(tc.tile_pool(name="small", bufs=1))
        bnpack = small.tile([128, 4], F32, tag="bnpack", name="bnpack")
        cin = dram.tile([128, 4], F32, name="cin")
        cout = dram.tile([128, 4], F32, name="cout")
        rhss = [x1[0], x1[1], x2[0], x2[1]]
        for mc in range(2):
            for n8 in range(8):
                sl = slice(512 * n8, 512 * (n8 + 1))
                ps = psH.tile([128, 512], F32, tag="fups", name="fups")
                for kc in range(4):
                    nc.tensor.matmul(ps[:], wfu_sb[:, kc, mc, :],
                                     rhss[kc][:, sl],
                                     start=(kc == 0), stop=(kc == 3))
                nc.scalar.activation(out_cm[mc][:, sl], ps[:], ACT.Relu,
                                     bias=bfu2[:, mc:mc + 1])
                nc.vector.tensor_tensor(out_cm[mc][:, sl], out_cm[mc][:, sl],
                                        xT_cm[mc][:, sl], op=ALU.add)
            st = small.tile([128, 8, 6], F32, tag="bnst", name="bnst")
            for n8 in range(8):
                nc.vector.bn_stats(st[:, n8, :], out_cm[mc][:, 512 * n8:512 * (n8 + 1)])
            ag = small.tile([128, 2], F32, tag="bnag", name="bnag")
            nc.vector.bn_aggr(ag[:], st[:])
            nc.vector.tensor_scalar(bnpack[:, 2 * mc:2 * mc + 1], ag[:, 0:1],
                                    float(L), None, op0=ALU.mult)
            sq = small.tile([128, 1], F32, tag="bnsq", name="bnsq")
            nc.vector.tensor_tensor(sq[:], ag[:, 0:1], ag[:, 0:1], op=ALU.mult)
            nc.vector.tensor_tensor(sq[:], sq[:], ag[:, 1:2], op=ALU.add)
            nc.vector.tensor_scalar(bnpack[:, 2 * mc + 1:2 * mc + 2], sq[:],
                                    float(L), None, op0=ALU.mult)
        # one packed AllReduce for both halves' (sum, sumsq)
        nc.sync.dma_start(cin[:], bnpack[:])
        nc.gpsimd.collective_compute(
            "AllReduce", ALU.add,
            replica_groups=[list(range(n_cores))],
            ins=[cin.opt()], outs=[cout.opt()])
        gs_sb = small.tile([128, 4], F32, tag="gsb", name="gs_sb")
        nc.sync.dma_start(gs_sb[:], cout[:])
        NTOT = float(n_cores * L)
        scale = small.tile([128, 2], F32, tag="scale", name="scale")
        shift = small.tile([128, 2], F32, tag="shift", name="shift")
        mean = small.tile([128, 2], F32, tag="mean", name="mean")
        var = small.tile([128, 2], F32, tag="var", name="var")
        for cc in range(2):
            nc.vector.tensor_scalar(mean[:, cc:cc + 1], gs_sb[:, 2 * cc:2 * cc + 1],
                                    1.0 / NTOT, None, op0=ALU.mult)
            nc.vector.tensor_scalar(var[:, cc:cc + 1], gs_sb[:, 2 * cc + 1:2 * cc + 2],
                                    1.0 / NTOT, None, op0=ALU.mult)
        msq = small.tile([128, 2], F32, tag="msq", name="msq")
        nc.vector.tensor_tensor(msq[:], mean[:], mean[:], op=ALU.mult)
        nc.vector.tensor_tensor(var[:], var[:], msq[:], op=ALU.subtract)
        rs = small.tile([128, 2], F32, tag="rs", name="rs")
        nc.vector.tensor_scalar(var[:], var[:], float(EPS), None, op0=ALU.add)
        nc.scalar.activation(rs[:], var[:], ACT.Sqrt)
        nc.vector.reciprocal(rs[:], rs[:])
        nc.vector.tensor_tensor(scale[:], gamma2[:], rs[:], op=ALU.mult)
        nc.vector.tensor_tensor(shift[:], mean[:], scale[:], op=ALU.mult)
        nc.vector.tensor_tensor(shift[:], beta2[:], shift[:], op=ALU.subtract)

        if DEBUG2:
            nc.gpsimd.dma_start(
                d["dbg_vt"][:].rearrange("p (j c) -> p j c", c=C), vT[:])
            nc.gpsimd.dma_start(d["dbg_wtap"][:], W_tap[:])
            nc.gpsimd.dma_start(
                d["dbg_wtm"][:].rearrange("p (j t) -> p j t", t=25), W_tm[:])
            nc.gpsimd.dma_start(d["dbg_ae"][:], AE[:])
            nc.gpsimd.dma_start(d["dbg_gs"][:], d["gs"][0:NCHUNK * GJ2])
            nc.gpsimd.dma_start(
                d["dbg_out"][:].rearrange("p (m l) -> p m l", l=L), out_all[:])
            for cc in range(2):
                nc.gpsimd.dma_start(d["dbg_xcm"][:, L * cc:L * (cc + 1)],
                                    xT_cm[cc][:])

        # normalize into the dead x1 tiles (bf16), DMA out c-major
        # (host un-transposes and upcasts)
        for n8 in range(8):
            sl = slice(512 * n8, 512 * (n8 + 1))
            for cc in range(2):
                if cc == 0:
                    nc.vector.tensor_scalar(x1[0][:, sl], out_cm[0][:, sl],
                                            scale[:, 0:1], shift[:, 0:1],
                                            op0=ALU.mult, op1=ALU.add)
                else:
                    nc.scalar.activation(x1[1][:, sl], out_cm[1][:, sl],
                                         ACT.Identity, bias=shift[:, 1:2],
                                         scale=scale[:, 1:2])
                eng = nc.sync if (n8 % 2 == 0) else nc.scalar
                eng.dma_start(d["y"][128 * cc:128 * (cc + 1), sl],
                              x1[cc][:, sl])
        cmH.__exit__(None, None, None)


_CACHE = {}


def _get_program(n_cores=N_CORES):
    key = n_cores
    if key not in _CACHE:
        nc = bacc.Bacc("TRN2", target_bir_lowering=False, debug=False,
                       num_devices=n_cores)
        build(nc, n_cores)
        nc.compile()
        _CACHE[key] = nc
    return _CACHE[key]


_CONSTS = None


def make_in_map(inputs, b):
    global _CONSTS
    if _CONSTS is None:
        _CONSTS = host_consts()
    import ml_dtypes
    # pre-permute to the transposed-grid token order l' = w*64 + h
    xbf = np.ascontiguousarray(
        np.asarray(inputs["x"][b]).transpose(1, 0, 2).reshape(L, C)
    ).astype(ml_dtypes.bfloat16)
    return {
        "xbf": xbf,
        "wv": np.ascontiguousarray(inputs["Wv"], np.float32),
        "bv": np.ascontiguousarray(np.asarray(inputs["bv"]).reshape(1, C), np.float32),
        "wa": np.ascontiguousarray(inputs["Wa"], np.float32),
        "ba": np.ascontiguousarray(np.asarray(inputs["ba"]).reshape(81, 1), np.float32),
        "wfu": np.ascontiguousarray(inputs["Wfu"], np.float32),
        "bfu2": np.ascontiguousarray(
            np.asarray(inputs["bfu"]).reshape(2, 128).T, np.float32),
        "gamma2": np.ascontiguousarray(
            np.asarray(inputs["gamma"]).reshape(2, 128).T, np.float32),
        "beta2": np.ascontiguousarray(
            np.asarray(inputs["beta"]).reshape(2, 128).T, np.float32),
        **_CONSTS,
    }


def postprocess(yarr):
    """[256, L] c-major, l' = w*64+h  ->  [H, W, C] in the reference frame."""
    return np.asarray(yarr, np.float32).reshape(C, L).T.reshape(H, W, C)


def kernel(**inputs):
    nc = _get_program()
    in_maps = [make_in_map(inputs, b) for b in range(B)]
    res = run_bass_kernel_spmd(nc, in_maps, list(range(N_CORES)))
    out = np.stack([postprocess(res.results[b]["y"]) for b in range(B)])
    return out.astype(np.float32)

